# revision 21
# baseline (speedup 1.0000x reference)
"""Causal multi-head attention block (GPT-style) on 8 TRN2 NeuronCores.

Sharding: core (b, g) = batch b in {0,1} x head-group g in {0..3} (4 heads of
dh=64 each). Megatron-style: each core computes q/k/v projections for its 256
channels, attention for its 4 heads, and a partial c_proj using its 256 rows of
W_proj. Host sums the 4 partial projections per batch (+ bias terms).

fp8(e4m3) strategy, validated against the 2e-2 absmax gate (measured ~1.5e-2):
  - qk projection: DoubleRow matmul; stationary planes = (W1, W2) residual
    pair at scale 32 (W ~ (W1+W2)/32); moving planes = (x8, x8) via stride-0
    broadcast_to. 2x over f32r.
  - scores: DoubleRow zero-plane form; plane 1 of q8/k8 tiles is a zeroed
    upper half, so each instr computes k8^T q8 at 0.5 cyc/col. 2x.
  - exp on ACT reads the score psum, applies the 1/sqrt(dh)=0.125 scale inside
    the activation, and writes u directly as fp8.
  - av: DoubleRow; stationary planes = (v1, v2) fp8 residual pair of the
    bf16-projected v (plain fp8 v fails the gate); moving planes = (u, u) via
    stride-0 broadcast_to. 2x.
  - v projection runs in bf16 (x and W_v loaded as bf16 - halves the largest
    DMA transfers); c_proj stays f32r.
DMAs are batched into ~30 large transfers: the serial HWDGE device charges
625ns per DMA and SP.SEQ ~650ns, which capped the old 116-DMA schedule at
~131us regardless of engine utilization.
Engine placement: ACT does only exp (the bottleneck); DVE does the psum->fp8
casts, softmax normalization, and c_proj psum evacuation; Pool (no PSUM
access) does the causal triangle masks on u and the partition broadcasts.
"""

import sys

try:
    import concourse  # noqa: F401
except ImportError:
    sys.path.insert(0, "/opt/trn_rl_repo")

from contextlib import ExitStack

import numpy as np
import ml_dtypes

import concourse.tile as tile
from concourse import bacc, mybir
from concourse.bass_utils import run_bass_kernel_spmd

F32 = mybir.dt.float32
F32R = mybir.dt.float32r
BF16 = mybir.dt.bfloat16
F8 = mybir.dt.float8e4
F16 = mybir.dt.float16
NP8 = ml_dtypes.float8_e4m3
NPBF = ml_dtypes.bfloat16
EXP = mybir.ActivationFunctionType.Exp
MUL = mybir.AluOpType.mult
ADD = mybir.AluOpType.add
SUB = mybir.AluOpType.subtract
DR = mybir.MatmulPerfMode.DoubleRow

B, T, D = 2, 2048, 1024
HG, DH = 4, 64          # heads per core, head dim
CQK = 512               # q+k channels per core
CV = 256                # v channels per core
KT = D // 128           # contraction tiles of the projections
KA, KB = 2, 6           # k-tile split of the x/w loads (startup pipelining)
TS = 512                # t-slice width
NTS = T // TS
NT128 = T // 128
SW = 32.0               # fp8 scale of the W_qk residual pair
# Vt per j-block: [resid (2, stride 320), head (4) x (64 v + 1 ones)].
# The DoubleRow ldweights plane stride must be a multiple of 64, so the
# resid-plane stride is padded 260 -> 320.
VS = 320                # resid-plane stride inside a j-block
VB = 2 * VS             # 640 bytes per j-block in Vt


def interleave(primary, filler, back=0.45):
    """Merge filler among primary units, weighted toward the back where the
    ACT pipeline debt is largest."""
    if not filler:
        return list(primary)
    n = len(primary)
    out = []
    fi = 0
    for i, p in enumerate(primary):
        out.append(p)
        x = (i + 1) / n
        want = ((1 - back) * x + back * x * x) * len(filler)
        while fi < len(filler) and fi + 1 <= want:
            out.append(filler[fi])
            fi += 1
    out.extend(filler[fi:])
    return out


def build():
    nc = bacc.Bacc(None)

    xT_in = nc.dram_tensor("xT", [NTS, 128, KT * TS], BF16, kind="ExternalInput")
    xT8_in = nc.dram_tensor("xT8", [NTS, 128, KT * TS], F8, kind="ExternalInput")
    wqk_in = nc.dram_tensor("wqk", [128, KT * 2 * CQK], F8, kind="ExternalInput")
    wv_in = nc.dram_tensor("wv", [128, KT * CV], BF16, kind="ExternalInput")
    wp_in = nc.dram_tensor("wp", [128, 2 * D], F32R, kind="ExternalInput")
    bias_in = nc.dram_tensor("bqk", [128, 4], F32, kind="ExternalInput")
    mask_in = nc.dram_tensor("mask", [128, 256], F32R, kind="ExternalInput")
    out_dram = nc.dram_tensor("out", [NT128, 128, 2 * TS], F16, kind="ExternalOutput")

    with ExitStack() as ctx:
        tc = ctx.enter_context(tile.TileContext(nc))

        const = ctx.enter_context(tc.tile_pool(name="const", bufs=1))
        big = ctx.enter_context(tc.tile_pool(name="big", bufs=1))
        upool = ctx.enter_context(tc.tile_pool(name="upool", bufs=8))
        rows = ctx.enter_context(tc.tile_pool(name="rows", bufs=2))
        rbcp = ctx.enter_context(tc.tile_pool(name="rbcp", bufs=3))
        outp = ctx.enter_context(tc.tile_pool(name="outp", bufs=4))
        xrp = ctx.enter_context(tc.tile_pool(name="xrp", bufs=2))
        xr8p = ctx.enter_context(tc.tile_pool(name="xr8p", bufs=2))

        ps_s = ctx.enter_context(tc.tile_pool(name="ps_s", bufs=2, space="PSUM"))
        ps_av = ctx.enter_context(tc.tile_pool(name="ps_av", bufs=2, space="PSUM"))
        ps_mm = ctx.enter_context(tc.tile_pool(name="ps_mm", bufs=2, space="PSUM"))

        # weight + first-slice DMAs, startup-critical first: the first
        # qk projections need wqk[k<2], x8[k<2] and the bias; v_group(0,0)
        # needs wv + xr(0) soon after.
        wqka = const.tile([128, KA * 2 * CQK], F8, tag="wqka")
        nc.sync.dma_start(out=wqka[:], in_=wqk_in[:, 0:KA * 2 * CQK])
        xr, xr8 = {}, {}

        def load_x8(ts):
            x8a = xr8p.tile([128, KA * TS], F8, tag="xr8a", name=f"xr8a_{ts}")
            nc.sync.dma_start(out=x8a[:], in_=xT8_in[ts][:, 0:KA * TS])
            xr8[ts, 0] = x8a
            x8b = xr8p.tile([128, KB * TS], F8, tag="xr8b", name=f"xr8b_{ts}")
            nc.sync.dma_start(out=x8b[:], in_=xT8_in[ts][:, KA * TS:])
            xr8[ts, 1] = x8b

        def load_xbf(ts):
            x1a = xrp.tile([128, KA * TS], BF16, tag="xra", name=f"xra_{ts}")
            nc.sync.dma_start(out=x1a[:], in_=xT_in[ts][:, 0:KA * TS])
            xr[ts, 0] = x1a
            x1b = xrp.tile([128, KB * TS], BF16, tag="xrb", name=f"xrb_{ts}")
            nc.sync.dma_start(out=x1b[:], in_=xT_in[ts][:, KA * TS:])
            xr[ts, 1] = x1b

        def load_xr(ts):
            def unit():
                load_x8(ts)
                load_xbf(ts)
            return unit

        x8a0 = xr8p.tile([128, KA * TS], F8, tag="xr8a", name="xr8a_0")
        nc.sync.dma_start(out=x8a0[:], in_=xT8_in[0][:, 0:KA * TS])
        xr8[0, 0] = x8a0
        wqkb = const.tile([128, KB * 2 * CQK], F8, tag="wqkb")
        nc.sync.dma_start(out=wqkb[:], in_=wqk_in[:, KA * 2 * CQK:])
        x8b0 = xr8p.tile([128, KB * TS], F8, tag="xr8b", name="xr8b_0")
        nc.sync.dma_start(out=x8b0[:], in_=xT8_in[0][:, KA * TS:])
        xr8[0, 1] = x8b0
        bias_sb = const.tile([128, 4], F32, tag="bias")
        nc.sync.dma_start(out=bias_sb[:], in_=bias_in[:])
        wv = const.tile([128, KT * CV], BF16, tag="wv")
        nc.sync.dma_start(out=wv[:], in_=wv_in[:])
        tri = const.tile([128, 256], F32R, tag="tri")  # [zeros(128) | tri(128)]
        nc.sync.dma_start(out=tri[:], in_=mask_in[:])
        load_xbf(0)
        wp = const.tile([128, 2 * D], F32R, tag="wp")
        nc.sync.dma_start(out=wp[:], in_=wp_in[:])

        def xr8_slice(ts, k):
            if k < KA:
                return xr8[ts, 0][:, k * TS:(k + 1) * TS]
            return xr8[ts, 1][:, (k - KA) * TS:(k - KA + 1) * TS]

        def xr_slice(ts, k, lo, hi):
            if k < KA:
                return xr[ts, 0][:, k * TS + lo:k * TS + hi]
            return xr[ts, 1][:, (k - KA) * TS + lo:(k - KA) * TS + hi]

        import dataclasses

        def _plane(t, off, stride=CQK, m=128, p0=0, pk=128):
            """Stationary DoubleRow AP [pk, 2(stride), m] at free offset off.
            Built from a 1-element slice so tile dep-tracking sees tile t;
            the stride must be a multiple of 64 (dual-fp8 ldweights rule)."""
            ap = t[p0:p0 + pk, off:off + 1].unsqueeze(1)
            return dataclasses.replace(
                ap, ap=[ap.ap[0], (stride, 2), (1, m)])

        ones128 = const.tile([128, HG], F32, tag="ones128")
        nc.vector.memset(ones128[:], 1.0)

        # persistent intermediates
        # q8/k8 per (ct, ts): [128, 1024] fp8; [0:512) data, [512:1024) zeros
        # (the zeros feed the unused DoubleRow plane). ct 0/1 = q of head-pair
        # 0/1, ct 2/3 = k of head-pair 0/1.
        qkT = {(ct, ts): big.tile([128, 2 * TS], F8, tag=f"qkT{ct}_{ts}",
                                  name=f"qkT{ct}_{ts}")
               for ct in range(4) for ts in range(NTS)}
        Vt = [big.tile([128, 4 * VB], F8, tag=f"Vt{ts}", name=f"Vt{ts}")
              for ts in range(NTS)]
        aT = {(hp, gi): big.tile([128, TS], F32R, tag=f"aT{hp}_{gi}",
                                 name=f"aT{hp}_{gi}")
              for hp in range(2) for gi in range(NTS)}

        # zero the DoubleRow upper halves once; set the v ones/zeros columns.
        # Order matters: slice-0 consumers run first, so init ts=0 tiles
        # first (qkT(0,0) fully - it doubles as the PE warmup operand).
        def init_vt(ts):
            v5 = Vt[ts][:].rearrange("p (s r q) -> p s r q", s=4, r=2)
            for r in range(2):
                ve = v5[:, :, r, 0:HG * (DH + 1)].rearrange(
                    "p s (h e) -> p s h e", e=DH + 1)
                if r == 0:
                    nc.gpsimd.tensor_copy(
                        ve[:, :, :, DH],
                        ones128[:].unsqueeze(1).broadcast_to((128, 4, HG)))
                else:
                    nc.gpsimd.memset(ve[:, :, :, DH], 0.0)

        nc.gpsimd.memset(qkT[0, 0][:], 0.0)
        for ct in (2, 1, 3):
            nc.gpsimd.memset(qkT[ct, 0][:, TS:], 0.0)
        init_vt(0)

        # PE warmup: the tensor engine runs at reduced clock for its first
        # ~3us of execution. Burn the p-state ramp on zero matmuls while the
        # first x8/wqk DMAs are still in flight.
        wps = ps_s.tile([128, 2 * TS], F32, tag="ss", name="warm")
        wmov = qkT[0, 0][:, 0:TS].unsqueeze(1).broadcast_to((128, 2, TS))
        for i in range(7):
            nc.tensor.matmul(wps[:, 0:TS], _plane(qkT[0, 0], 0, stride=TS),
                             wmov, start=True, stop=True, perf_mode=DR)

        for ts in range(1, NTS):
            for ct in range(4):
                nc.gpsimd.memset(qkT[ct, ts][:, TS:], 0.0)
            init_vt(ts)

        def qk_group(ts, ct):
            """DoubleRow projection of 128 q/k channels for slice ts:
            psum = x8 @ (W1+W2) = 32*qk; DVE casts to fp8 with 1/32 (+bias)."""
            def unit():
                ps = ps_mm.tile([128, TS], F32, tag="mm", name=f"qk_{ts}_{ct}")
                for k in range(KT):
                    t, kk = (wqka, k) if k < KA else (wqkb, k - KA)
                    wst = _plane(t, kk * 2 * CQK + ct * 128)
                    xb = xr8_slice(ts, k).unsqueeze(1).broadcast_to((128, 2, TS))
                    nc.tensor.matmul(ps[:], wst, xb,
                                     start=(k == 0), stop=(k == KT - 1),
                                     perf_mode=DR)
                nc.vector.tensor_scalar(
                    qkT[ct, ts][:, 0:TS], ps[:],
                    1.0 / SW, bias_sb[:, ct:ct + 1], op0=MUL, op1=ADD)
            return unit

        def v_group(ts, sub):
            """bf16 v projection of j-block sub of slice ts, then fp8
            residual-pair cast into Vt[ts]."""
            def unit():
                ps = ps_mm.tile([128, CV], F32, tag="mm", name=f"v_{ts}_{sub}")
                for k in range(KT):
                    nc.tensor.matmul(ps[:],
                                     xr_slice(ts, k, sub * 128, (sub + 1) * 128),
                                     wv[:, k * CV:(k + 1) * CV],
                                     start=(k == 0), stop=(k == KT - 1))
                v5 = Vt[ts][:].rearrange("p (s r q) -> p s r q", s=4, r=2)
                v0 = v5[:, sub, 0, 0:HG * (DH + 1)].rearrange(
                    "p (h e) -> p h e", e=DH + 1)[:, :, 0:DH]
                v1 = v5[:, sub, 1, 0:HG * (DH + 1)].rearrange(
                    "p (h e) -> p h e", e=DH + 1)[:, :, 0:DH]
                p3 = ps[:].rearrange("p (h e) -> p h e", e=DH)
                nc.vector.tensor_copy(v0, p3)
                nc.vector.tensor_tensor(v1, p3, v0, op=SUB)
            return unit

        # ---- attention ----
        sstiles = {}
        utiles = {}
        avtiles = {}

        def att_sc(gi, h, p):
            """Scores for j-tile pair (2p, 2p+1): two DoubleRow zero-plane
            matmuls into one [128, 2, TS] psum tile."""
            npairs = 2 * (gi + 1)
            c0 = 256 if p == npairs - 1 else 0
            hp, p0 = h // 2, 64 * (h % 2)

            def unit():
                ss = ps_s.tile([128, 2 * TS], F32, tag="ss",
                               name=f"ss_{gi}_{h}_{p}")
                sstiles[gi, h, p] = ss
                s3 = ss[:].rearrange("q (j i) -> q j i", j=2)
                qm = qkT[hp, gi][p0:p0 + 64, :].rearrange(
                    "q (r i) -> q r i", r=2)[:, :, c0:TS]
                for jj in range(2):
                    jt = 2 * p + jj
                    jts, jo = jt // 4, (jt % 4) * 128
                    km = _plane(qkT[2 + hp, jts], jo, stride=TS, m=128,
                                p0=p0, pk=64)
                    nc.tensor.matmul(s3[:, jj, c0:TS], km, qm,
                                     start=True, stop=True, perf_mode=DR)
            return unit

        def att_exp(gi, h, p):
            """exp of the score pair -> fp8 u tile; Pool masks the causal
            boundary on the diagonal pairs."""
            npairs = 2 * (gi + 1)
            c0 = 256 if p == npairs - 1 else 0

            def unit():
                ss = sstiles.pop((gi, h, p))
                u = upool.tile([128, 2 * TS], F8, tag="u",
                               name=f"u_{gi}_{h}_{p}")
                utiles[gi, h, p] = u
                u3 = u[:].rearrange("q (j i) -> q j i", j=2)
                s3 = ss[:].rearrange("q (j i) -> q j i", j=2)
                nc.scalar.activation(u3[:, :, c0:TS], s3[:, :, c0:TS], EXP,
                                     scale=0.125)
                if p == 2 * gi:        # diagonal pair (d = 0, 128)
                    nc.gpsimd.tensor_tensor(
                        u3[:, 0, 0:128], u3[:, 0, 0:128],
                        tri[:, 128:256], op=MUL)
                    nc.gpsimd.tensor_tensor(
                        u3[:, 1, 0:256], u3[:, 1, 0:256],
                        tri[:, 0:256], op=MUL)
                elif p == 2 * gi + 1:  # diagonal pair (d = 256, 384)
                    nc.gpsimd.tensor_tensor(
                        u3[:, 0, 256:384], u3[:, 0, 256:384],
                        tri[:, 128:256], op=MUL)
                    nc.gpsimd.tensor_tensor(
                        u3[:, 1, 256:512], u3[:, 1, 256:512],
                        tri[:, 0:256], op=MUL)
            return unit

        def att_av(gi, h, p):
            """attn-out accumulation for the pair: two DoubleRow matmuls with
            v-residual stationary planes and stride-0 u moving planes."""
            npairs = 2 * (gi + 1)

            def unit():
                if p == 0:
                    avtiles[gi, h] = ps_av.tile([DH + 1, TS], F32, tag="av",
                                                name=f"av_{gi}_{h}")
                av = avtiles[gi, h]
                u = utiles[gi, h, p]
                if p == npairs - 1:
                    utiles.pop((gi, h, p))
                u3 = u[:].rearrange("q (j i) -> q j i", j=2)
                for jj in range(2):
                    jt = 2 * p + jj
                    d = jt * 128 - gi * TS
                    c0 = max(d, 0)
                    jts, jb = jt // 4, jt % 4
                    vst = _plane(Vt[jts], jb * VB + h * (DH + 1),
                                 stride=VS, m=DH + 1)
                    um = u3[:, jj, c0:TS].unsqueeze(1).broadcast_to(
                        (128, 2, TS - c0))
                    nc.tensor.matmul(av[:, c0:TS], vst, um,
                                     start=(jt == 0), stop=(jt == 2 * npairs - 1),
                                     perf_mode=DR)
            return unit

        def att_norm(gi, h):
            """softmax normalization: 1/denominator broadcast-multiplied into
            aT, straight from the av psum tile."""
            hp, p0 = h // 2, 64 * (h % 2)

            def unit():
                av = avtiles.pop((gi, h))
                r = rows.tile([1, TS], F32, tag="r", name=f"r_{gi}_{h}")
                nc.vector.reciprocal(r[:], av[DH:DH + 1, :])
                rbc = rbcp.tile([DH, TS], F32, tag="rbc", name=f"rbc_{gi}_{h}")
                nc.gpsimd.partition_broadcast(rbc[:], r[:])
                nc.vector.tensor_tensor(
                    aT[hp, gi][p0:p0 + 64, :], av[0:DH, :], rbc[:], op=MUL)
            return unit

        def proj_unit(tt, tail=False):
            gi = tt // 4

            def unit():
                o = outp.tile([128, 2 * TS], F16, tag="o", name=f"o_{tt}")
                if tail:
                    # attention psum pools are free by now; one 2-bank tile
                    # holds both nt accumulation groups -> single merged copy
                    ps2 = ps_s.tile([128, 2 * TS], F32, tag="ss",
                                    name=f"pj_{tt}")
                    for nt in range(2):
                        for c in range(2):
                            nc.tensor.matmul(
                                ps2[:, nt * TS:(nt + 1) * TS],
                                aT[c, gi][:, (tt % 4) * 128:(tt % 4 + 1) * 128],
                                wp[:, c * D + nt * TS:c * D + (nt + 1) * TS],
                                start=(c == 0), stop=(c == 1))
                    nc.vector.tensor_copy(o[:], ps2[:])
                else:
                    for nt in range(2):
                        ps = ps_mm.tile([128, TS], F32, tag="mm",
                                        name=f"pj_{tt}_{nt}")
                        for c in range(2):
                            nc.tensor.matmul(
                                ps[:],
                                aT[c, gi][:, (tt % 4) * 128:(tt % 4 + 1) * 128],
                                wp[:, c * D + nt * TS:c * D + (nt + 1) * TS],
                                start=(c == 0), stop=(c == 1))
                        nc.vector.tensor_copy(o[:, nt * TS:(nt + 1) * TS],
                                              ps[:])
                nc.sync.dma_start(out=out_dram[tt], in_=o[:])
            return unit

        def phase_a_units(ts):
            us = []
            if ts > 0:
                us.append(load_xr(ts))
            # all q/k groups first: they gate the next slice's attention
            for ct in (0, 2, 1, 3):
                us.append(qk_group(ts, ct))
            for sub in range(4):
                us.append(v_group(ts, sub))
            return us

        def attention_units(gi):
            """Flat pipeline over (h, pair): av trails its exp by `depth`
            slots so the ACT stream never drains."""
            npairs = 2 * (gi + 1)
            seq = [(h, p) for h in range(HG) for p in range(npairs)]
            depth = min(3, npairs)
            us = []
            for idx, (h, p) in enumerate(seq):
                us.append(att_sc(gi, h, p))
                us.append(att_exp(gi, h, p))
                if idx >= depth:
                    ph, pp = seq[idx - depth]
                    us.append(att_av(gi, ph, pp))
                    if pp == npairs - 1:
                        us.append(att_norm(gi, ph))
            for idx in range(len(seq) - depth, len(seq)):
                ph, pp = seq[idx]
                us.append(att_av(gi, ph, pp))
                if pp == npairs - 1:
                    us.append(att_norm(gi, ph))
            return us

        # slice 0: only the ct 0/2 projections must precede attention(0,h0).
        # The rest of phase A is injected into attention(0) at positions that
        # keep every producer emitted before its consumer (tile dependency
        # tracking is emission-order based): v_group(sub) before the first
        # att_av reading that j-block.
        qk_group(0, 0)()
        qk_group(0, 2)()

        def attention0_units():
            a = attention_units(0)
            # flat layout: [sc00,e00, sc01,e01, sc10,e10, av00, sc11,e11,
            #               av01, norm0, ...]
            inject = [(2, qk_group(0, 1)), (4, qk_group(0, 3)),
                      (6, v_group(0, 0)), (6, v_group(0, 1)),
                      (9, v_group(0, 2)), (9, v_group(0, 3))]
            for pos, u in reversed(inject):
                a.insert(pos, u)
            return a

        for gi in range(NTS):
            if gi < NTS - 1:
                filler = phase_a_units(gi + 1)
            else:
                filler = [proj_unit(tt) for tt in range(0, 12)]
            prim = attention0_units() if gi == 0 else attention_units(gi)
            for u in interleave(prim, filler, back=0.0):
                u()
        for tt in range(12, 16):
            proj_unit(tt, tail=True)()

    nc.finalize()
    return nc


_NC = None


def _get_nc():
    global _NC
    if _NC is None:
        _NC = build()
    return _NC


def _make_in_maps(x, W_attn, b_attn, W_proj):
    jj = np.arange(128, dtype=np.int64)[:, None]
    ii = np.arange(128, dtype=np.int64)[None, :]
    tri = (jj <= ii).astype(np.float32)
    mask = np.ascontiguousarray(
        np.concatenate([np.zeros((128, 128), np.float32), tri], axis=1))

    shards = []
    for g in range(4):
        q_cols = W_attn[:, g * CV:(g + 1) * CV]
        k_cols = W_attn[:, D + g * CV:D + (g + 1) * CV]
        wqk = np.concatenate([q_cols, k_cols], axis=1)          # [D, 512]
        w1 = (wqk * SW).astype(NP8)
        w2 = (wqk * SW - np.asarray(w1, np.float32)).astype(NP8)
        # [128, (k, resid, c)]
        wqk8 = np.stack([w1.reshape(KT, 128, CQK),
                         w2.reshape(KT, 128, CQK)], axis=2)     # [KT,128,2,512]
        wqk8 = np.ascontiguousarray(
            wqk8.transpose(1, 0, 2, 3).reshape(128, KT * 2 * CQK))
        wvg = np.ascontiguousarray(
            W_attn[:, 2 * D + g * CV:2 * D + (g + 1) * CV].reshape(
                KT, 128, CV).transpose(1, 0, 2).reshape(128, KT * CV)
        ).astype(NPBF)
        wpg = np.ascontiguousarray(
            W_proj[g * CV:(g + 1) * CV, :].reshape(2, 128, D).transpose(
                1, 0, 2).reshape(128, 2 * D))
        bq = b_attn[g * CV:(g + 1) * CV]
        bk = b_attn[D + g * CV:D + (g + 1) * CV]
        bqk = np.ascontiguousarray(
            np.concatenate([bq, bk]).reshape(4, 128).T).astype(np.float32)
        shards.append((wqk8, wvg, wpg, bqk))

    in_maps = []
    for b in range(B):
        xt = x[b].T.reshape(KT, 128, NTS, TS)                   # [k, p, ts, t]
        xT = np.ascontiguousarray(
            xt.transpose(2, 1, 0, 3).reshape(NTS, 128, KT * TS))
        xT8 = xT.astype(NP8)
        xTb = xT.astype(NPBF)
        for g in range(4):
            wqk8, wvg, wpg, bqk = shards[g]
            in_maps.append({
                "xT": xTb, "xT8": xT8, "wqk": wqk8, "wv": wvg, "wp": wpg,
                "bqk": bqk, "mask": mask,
            })
    return in_maps


def run(inputs, trace=False):
    x = np.asarray(inputs["x"], dtype=np.float32)
    W_attn = np.asarray(inputs["W_attn"], dtype=np.float32)
    b_attn = np.asarray(inputs["b_attn"], dtype=np.float32)
    W_proj = np.asarray(inputs["W_proj"], dtype=np.float32)
    b_proj = np.asarray(inputs["b_proj"], dtype=np.float32)

    nc = _get_nc()
    in_maps = _make_in_maps(x, W_attn, b_attn, W_proj)
    res = run_bass_kernel_spmd(nc, in_maps, list(range(8)), trace=trace)

    out = np.zeros((B, T, D), dtype=np.float32)
    for b in range(B):
        for g in range(4):
            out[b] += res.results[b * 4 + g]["out"].reshape(T, D).astype(np.float32)
    # v-bias contributes a constant shift through the value path; b_proj too.
    const = b_attn[2 * D:3 * D] @ W_proj + b_proj
    out += const[None, None, :].astype(np.float32)
    return out, res


def kernel(**inputs):
    out, _ = run(inputs, trace=False)
    return out


# revision 23
# speedup vs baseline: 1.0029x; 1.0029x over previous
"""Causal multi-head attention block (GPT-style) on 8 TRN2 NeuronCores.

Sharding: core (b, g) = batch b in {0,1} x head-group g in {0..3} (4 heads of
dh=64 each). Megatron-style: each core computes q/k/v projections for its 256
channels, attention for its 4 heads, and a partial c_proj using its 256 rows of
W_proj. Host sums the 4 partial projections per batch (+ bias terms).

fp8(e4m3) strategy, validated against the 2e-2 absmax gate (measured ~1.5e-2):
  - qk projection: DoubleRow matmul; stationary planes = (W1, W2) residual
    pair at scale 32 (W ~ (W1+W2)/32); moving planes = (x8, x8) via stride-0
    broadcast_to. 2x over f32r.
  - scores: DoubleRow zero-plane form; plane 1 of q8/k8 tiles is a zeroed
    upper half, so each instr computes k8^T q8 at 0.5 cyc/col. 2x.
  - exp on ACT reads the score psum, applies the 1/sqrt(dh)=0.125 scale inside
    the activation, and writes u directly as fp8.
  - av: DoubleRow; stationary planes = (v1, v2) fp8 residual pair of the
    bf16-projected v (plain fp8 v fails the gate); moving planes = (u, u) via
    stride-0 broadcast_to. 2x.
  - v projection runs in bf16 (x and W_v loaded as bf16 - halves the largest
    DMA transfers); c_proj stays f32r.
DMAs are batched into ~30 large transfers: the serial HWDGE device charges
625ns per DMA and SP.SEQ ~650ns, which capped the old 116-DMA schedule at
~131us regardless of engine utilization.
Engine placement: ACT does only exp (the bottleneck); DVE does the psum->fp8
casts, softmax normalization, and c_proj psum evacuation; Pool (no PSUM
access) does the causal triangle masks on u and the partition broadcasts.
"""

import sys

try:
    import concourse  # noqa: F401
except ImportError:
    sys.path.insert(0, "/opt/trn_rl_repo")

from contextlib import ExitStack

import numpy as np
import ml_dtypes

import concourse.tile as tile
from concourse import bacc, mybir
from concourse.bass_utils import run_bass_kernel_spmd

F32 = mybir.dt.float32
F32R = mybir.dt.float32r
BF16 = mybir.dt.bfloat16
F8 = mybir.dt.float8e4
F16 = mybir.dt.float16
NP8 = ml_dtypes.float8_e4m3
NPBF = ml_dtypes.bfloat16
EXP = mybir.ActivationFunctionType.Exp
MUL = mybir.AluOpType.mult
ADD = mybir.AluOpType.add
SUB = mybir.AluOpType.subtract
DR = mybir.MatmulPerfMode.DoubleRow

B, T, D = 2, 2048, 1024
HG, DH = 4, 64          # heads per core, head dim
CQK = 512               # q+k channels per core
CV = 256                # v channels per core
KT = D // 128           # contraction tiles of the projections
KA, KB = 2, 6           # k-tile split of the x/w loads (startup pipelining)
TS = 512                # t-slice width
NTS = T // TS
NT128 = T // 128
SW = 32.0               # fp8 scale of the W_qk residual pair
# Vt per j-block: [resid (2, stride 320), head (4) x (64 v + 1 ones)].
# The DoubleRow ldweights plane stride must be a multiple of 64, so the
# resid-plane stride is padded 260 -> 320.
VS = 320                # resid-plane stride inside a j-block
VB = 2 * VS             # 640 bytes per j-block in Vt


def interleave(primary, filler, back=0.45):
    """Merge filler among primary units, weighted toward the back where the
    ACT pipeline debt is largest."""
    if not filler:
        return list(primary)
    n = len(primary)
    out = []
    fi = 0
    for i, p in enumerate(primary):
        out.append(p)
        x = (i + 1) / n
        want = ((1 - back) * x + back * x * x) * len(filler)
        while fi < len(filler) and fi + 1 <= want:
            out.append(filler[fi])
            fi += 1
    out.extend(filler[fi:])
    return out


def build():
    nc = bacc.Bacc(None)

    xT_in = nc.dram_tensor("xT", [NTS, 128, KT * TS], BF16, kind="ExternalInput")
    xT8_in = nc.dram_tensor("xT8", [NTS, 128, KT * TS], F8, kind="ExternalInput")
    wqk_in = nc.dram_tensor("wqk", [128, KT * 2 * CQK], F8, kind="ExternalInput")
    wv_in = nc.dram_tensor("wv", [128, KT * CV], BF16, kind="ExternalInput")
    wp_in = nc.dram_tensor("wp", [128, 2 * D], F32R, kind="ExternalInput")
    bias_in = nc.dram_tensor("bqk", [128, 4], F32, kind="ExternalInput")
    mask_in = nc.dram_tensor("mask", [128, 256], F32R, kind="ExternalInput")
    out_dram = nc.dram_tensor("out", [NT128, 128, 2 * TS], F16, kind="ExternalOutput")

    with ExitStack() as ctx:
        tc = ctx.enter_context(tile.TileContext(nc))

        const = ctx.enter_context(tc.tile_pool(name="const", bufs=1))
        big = ctx.enter_context(tc.tile_pool(name="big", bufs=1))
        upool = ctx.enter_context(tc.tile_pool(name="upool", bufs=8))
        rows = ctx.enter_context(tc.tile_pool(name="rows", bufs=2))
        rbcp = ctx.enter_context(tc.tile_pool(name="rbcp", bufs=3))
        outp = ctx.enter_context(tc.tile_pool(name="outp", bufs=4))
        xrp = ctx.enter_context(tc.tile_pool(name="xrp", bufs=2))
        xr8p = ctx.enter_context(tc.tile_pool(name="xr8p", bufs=2))

        ps_s = ctx.enter_context(tc.tile_pool(name="ps_s", bufs=2, space="PSUM"))
        ps_av = ctx.enter_context(tc.tile_pool(name="ps_av", bufs=2, space="PSUM"))
        ps_mm = ctx.enter_context(tc.tile_pool(name="ps_mm", bufs=2, space="PSUM"))

        # weight + first-slice DMAs, startup-critical first: the first
        # qk projections need wqk[k<2], x8[k<2] and the bias; v_group(0,0)
        # needs wv + xr(0) soon after.
        wqka = const.tile([128, KA * 2 * CQK], F8, tag="wqka")
        nc.sync.dma_start(out=wqka[:], in_=wqk_in[:, 0:KA * 2 * CQK])
        xr, xr8 = {}, {}

        def load_x8(ts):
            x8a = xr8p.tile([128, KA * TS], F8, tag="xr8a", name=f"xr8a_{ts}")
            nc.sync.dma_start(out=x8a[:], in_=xT8_in[ts][:, 0:KA * TS])
            xr8[ts, 0] = x8a
            x8b = xr8p.tile([128, KB * TS], F8, tag="xr8b", name=f"xr8b_{ts}")
            nc.sync.dma_start(out=x8b[:], in_=xT8_in[ts][:, KA * TS:])
            xr8[ts, 1] = x8b

        def load_xbf(ts):
            x1a = xrp.tile([128, KA * TS], BF16, tag="xra", name=f"xra_{ts}")
            nc.sync.dma_start(out=x1a[:], in_=xT_in[ts][:, 0:KA * TS])
            xr[ts, 0] = x1a
            x1b = xrp.tile([128, KB * TS], BF16, tag="xrb", name=f"xrb_{ts}")
            nc.sync.dma_start(out=x1b[:], in_=xT_in[ts][:, KA * TS:])
            xr[ts, 1] = x1b

        def load_xr(ts):
            def unit():
                load_x8(ts)
                load_xbf(ts)
            return unit

        x8a0 = xr8p.tile([128, KA * TS], F8, tag="xr8a", name="xr8a_0")
        nc.sync.dma_start(out=x8a0[:], in_=xT8_in[0][:, 0:KA * TS])
        xr8[0, 0] = x8a0
        wqkb = const.tile([128, KB * 2 * CQK], F8, tag="wqkb")
        nc.sync.dma_start(out=wqkb[:], in_=wqk_in[:, KA * 2 * CQK:])
        x8b0 = xr8p.tile([128, KB * TS], F8, tag="xr8b", name="xr8b_0")
        nc.sync.dma_start(out=x8b0[:], in_=xT8_in[0][:, KA * TS:])
        xr8[0, 1] = x8b0
        bias_sb = const.tile([128, 4], F32, tag="bias")
        nc.sync.dma_start(out=bias_sb[:], in_=bias_in[:])
        wv = const.tile([128, KT * CV], BF16, tag="wv")
        nc.sync.dma_start(out=wv[:], in_=wv_in[:])
        tri = const.tile([128, 256], F32R, tag="tri")  # [zeros(128) | tri(128)]
        nc.sync.dma_start(out=tri[:], in_=mask_in[:])
        load_xbf(0)
        wp = const.tile([128, 2 * D], F32R, tag="wp")
        nc.sync.dma_start(out=wp[:], in_=wp_in[:])

        def xr8_slice(ts, k):
            if k < KA:
                return xr8[ts, 0][:, k * TS:(k + 1) * TS]
            return xr8[ts, 1][:, (k - KA) * TS:(k - KA + 1) * TS]

        def xr_slice(ts, k, lo, hi):
            if k < KA:
                return xr[ts, 0][:, k * TS + lo:k * TS + hi]
            return xr[ts, 1][:, (k - KA) * TS + lo:(k - KA) * TS + hi]

        import dataclasses

        def _plane(t, off, stride=CQK, m=128, p0=0, pk=128):
            """Stationary DoubleRow AP [pk, 2(stride), m] at free offset off.
            Built from a 1-element slice so tile dep-tracking sees tile t;
            the stride must be a multiple of 64 (dual-fp8 ldweights rule)."""
            ap = t[p0:p0 + pk, off:off + 1].unsqueeze(1)
            return dataclasses.replace(
                ap, ap=[ap.ap[0], (stride, 2), (1, m)])

        ones128 = const.tile([128, HG], F32, tag="ones128")
        nc.vector.memset(ones128[:], 1.0)

        # persistent intermediates
        # q8/k8 per (ct, ts): [128, 1024] fp8; [0:512) data, [512:1024) zeros
        # (the zeros feed the unused DoubleRow plane). ct 0/1 = q of head-pair
        # 0/1, ct 2/3 = k of head-pair 0/1.
        qkT = {(ct, ts): big.tile([128, 2 * TS], F8, tag=f"qkT{ct}_{ts}",
                                  name=f"qkT{ct}_{ts}")
               for ct in range(4) for ts in range(NTS)}
        Vt = [big.tile([128, 4 * VB], F8, tag=f"Vt{ts}", name=f"Vt{ts}")
              for ts in range(NTS)]
        aT = {(hp, gi): big.tile([128, TS], F32R, tag=f"aT{hp}_{gi}",
                                 name=f"aT{hp}_{gi}")
              for hp in range(2) for gi in range(NTS)}

        # zero the DoubleRow upper halves once; set the v ones/zeros columns.
        # Order matters: slice-0 consumers run first, so init ts=0 tiles
        # first (qkT(0,0) fully - it doubles as the PE warmup operand).
        def init_vt(ts):
            v5 = Vt[ts][:].rearrange("p (s r q) -> p s r q", s=4, r=2)
            for r in range(2):
                ve = v5[:, :, r, 0:HG * (DH + 1)].rearrange(
                    "p s (h e) -> p s h e", e=DH + 1)
                if r == 0:
                    nc.gpsimd.tensor_copy(
                        ve[:, :, :, DH],
                        ones128[:].unsqueeze(1).broadcast_to((128, 4, HG)))
                else:
                    nc.gpsimd.memset(ve[:, :, :, DH], 0.0)

        nc.gpsimd.memset(qkT[0, 0][:], 0.0)
        for ct in (2, 1, 3):
            nc.gpsimd.memset(qkT[ct, 0][:, TS:], 0.0)
        init_vt(0)

        # PE warmup: the tensor engine runs at reduced clock for its first
        # ~3us of execution. Burn the p-state ramp on zero matmuls while the
        # first x8/wqk DMAs are still in flight.
        wps = ps_s.tile([128, 2 * TS], F32, tag="ss", name="warm")
        wmov = qkT[0, 0][:, 0:TS].unsqueeze(1).broadcast_to((128, 2, TS))
        for i in range(7):
            nc.tensor.matmul(wps[:, 0:TS], _plane(qkT[0, 0], 0, stride=TS),
                             wmov, start=True, stop=True, perf_mode=DR)

        for ts in range(1, NTS):
            for ct in range(4):
                nc.gpsimd.memset(qkT[ct, ts][:, TS:], 0.0)
            init_vt(ts)

        def qk_group(ts, ct):
            """DoubleRow projection of 128 q/k channels for slice ts:
            psum = x8 @ (W1+W2) = 32*qk; DVE casts to fp8 with 1/32 (+bias)."""
            def unit():
                ps = ps_mm.tile([128, TS], F32, tag="mm", name=f"qk_{ts}_{ct}")
                for k in range(KT):
                    t, kk = (wqka, k) if k < KA else (wqkb, k - KA)
                    wst = _plane(t, kk * 2 * CQK + ct * 128)
                    xb = xr8_slice(ts, k).unsqueeze(1).broadcast_to((128, 2, TS))
                    nc.tensor.matmul(ps[:], wst, xb,
                                     start=(k == 0), stop=(k == KT - 1),
                                     perf_mode=DR)
                nc.vector.tensor_scalar(
                    qkT[ct, ts][:, 0:TS], ps[:],
                    1.0 / SW, bias_sb[:, ct:ct + 1], op0=MUL, op1=ADD)
            return unit

        def v_group(ts, sub):
            """bf16 v projection of j-block sub of slice ts, then fp8
            residual-pair cast into Vt[ts]."""
            def unit():
                ps = ps_mm.tile([128, CV], F32, tag="mm", name=f"v_{ts}_{sub}")
                for k in range(KT):
                    nc.tensor.matmul(ps[:],
                                     xr_slice(ts, k, sub * 128, (sub + 1) * 128),
                                     wv[:, k * CV:(k + 1) * CV],
                                     start=(k == 0), stop=(k == KT - 1))
                v5 = Vt[ts][:].rearrange("p (s r q) -> p s r q", s=4, r=2)
                v0 = v5[:, sub, 0, 0:HG * (DH + 1)].rearrange(
                    "p (h e) -> p h e", e=DH + 1)[:, :, 0:DH]
                v1 = v5[:, sub, 1, 0:HG * (DH + 1)].rearrange(
                    "p (h e) -> p h e", e=DH + 1)[:, :, 0:DH]
                p3 = ps[:].rearrange("p (h e) -> p h e", e=DH)
                nc.vector.tensor_copy(v0, p3)
                nc.vector.tensor_tensor(v1, p3, v0, op=SUB)
            return unit

        # ---- attention ----
        sstiles = {}
        utiles = {}
        avtiles = {}

        def att_sc(gi, h, p):
            """Scores for j-tile pair (2p, 2p+1): two DoubleRow zero-plane
            matmuls into one [128, 2, TS] psum tile."""
            npairs = 2 * (gi + 1)
            c0 = 256 if p == npairs - 1 else 0
            hp, p0 = h // 2, 64 * (h % 2)

            def unit():
                ss = ps_s.tile([128, 2 * TS], F32, tag="ss",
                               name=f"ss_{gi}_{h}_{p}")
                sstiles[gi, h, p] = ss
                s3 = ss[:].rearrange("q (j i) -> q j i", j=2)
                qm = qkT[hp, gi][p0:p0 + 64, :].rearrange(
                    "q (r i) -> q r i", r=2)[:, :, c0:TS]
                for jj in range(2):
                    jt = 2 * p + jj
                    jts, jo = jt // 4, (jt % 4) * 128
                    km = _plane(qkT[2 + hp, jts], jo, stride=TS, m=128,
                                p0=p0, pk=64)
                    nc.tensor.matmul(s3[:, jj, c0:TS], km, qm,
                                     start=True, stop=True, perf_mode=DR)
            return unit

        def att_exp(gi, h, p):
            """exp of the score pair -> fp8 u tile; Pool masks the causal
            boundary on the diagonal pairs."""
            npairs = 2 * (gi + 1)
            c0 = 256 if p == npairs - 1 else 0

            def unit():
                ss = sstiles.pop((gi, h, p))
                u = upool.tile([128, 2 * TS], F8, tag="u",
                               name=f"u_{gi}_{h}_{p}")
                utiles[gi, h, p] = u
                u3 = u[:].rearrange("q (j i) -> q j i", j=2)
                s3 = ss[:].rearrange("q (j i) -> q j i", j=2)
                nc.scalar.activation(u3[:, :, c0:TS], s3[:, :, c0:TS], EXP,
                                     scale=0.125)
                if p == 2 * gi:        # diagonal pair (d = 0, 128)
                    nc.gpsimd.tensor_tensor(
                        u3[:, 0, 0:128], u3[:, 0, 0:128],
                        tri[:, 128:256], op=MUL)
                    nc.gpsimd.tensor_tensor(
                        u3[:, 1, 0:256], u3[:, 1, 0:256],
                        tri[:, 0:256], op=MUL)
                elif p == 2 * gi + 1:  # diagonal pair (d = 256, 384)
                    nc.gpsimd.tensor_tensor(
                        u3[:, 0, 256:384], u3[:, 0, 256:384],
                        tri[:, 128:256], op=MUL)
                    nc.gpsimd.tensor_tensor(
                        u3[:, 1, 256:512], u3[:, 1, 256:512],
                        tri[:, 0:256], op=MUL)
            return unit

        def att_av(gi, h, p):
            """attn-out accumulation for the pair: two DoubleRow matmuls with
            v-residual stationary planes and stride-0 u moving planes."""
            npairs = 2 * (gi + 1)

            def unit():
                if p == 0:
                    avtiles[gi, h] = ps_av.tile([DH + 1, TS], F32, tag="av",
                                                name=f"av_{gi}_{h}")
                av = avtiles[gi, h]
                u = utiles[gi, h, p]
                if p == npairs - 1:
                    utiles.pop((gi, h, p))
                u3 = u[:].rearrange("q (j i) -> q j i", j=2)
                for jj in range(2):
                    jt = 2 * p + jj
                    d = jt * 128 - gi * TS
                    c0 = max(d, 0)
                    jts, jb = jt // 4, jt % 4
                    vst = _plane(Vt[jts], jb * VB + h * (DH + 1),
                                 stride=VS, m=DH + 1)
                    um = u3[:, jj, c0:TS].unsqueeze(1).broadcast_to(
                        (128, 2, TS - c0))
                    nc.tensor.matmul(av[:, c0:TS], vst, um,
                                     start=(jt == 0), stop=(jt == 2 * npairs - 1),
                                     perf_mode=DR)
            return unit

        def att_norm(gi, h):
            """softmax normalization: 1/denominator broadcast-multiplied into
            aT, straight from the av psum tile."""
            hp, p0 = h // 2, 64 * (h % 2)

            def unit():
                av = avtiles.pop((gi, h))
                r = rows.tile([1, TS], F32, tag="r", name=f"r_{gi}_{h}")
                nc.vector.reciprocal(r[:], av[DH:DH + 1, :])
                rbc = rbcp.tile([DH, TS], F32, tag="rbc", name=f"rbc_{gi}_{h}")
                nc.gpsimd.partition_broadcast(rbc[:], r[:])
                nc.vector.tensor_tensor(
                    aT[hp, gi][p0:p0 + 64, :], av[0:DH, :], rbc[:], op=MUL)
            return unit

        def proj_unit(tt, tail=False):
            gi = tt // 4

            def unit():
                o = outp.tile([128, 2 * TS], F16, tag="o", name=f"o_{tt}")
                if tail:
                    # attention psum pools are free by now; one 2-bank tile
                    # holds both nt accumulation groups -> single merged copy
                    ps2 = ps_s.tile([128, 2 * TS], F32, tag="ss",
                                    name=f"pj_{tt}")
                    for nt in range(2):
                        for c in range(2):
                            nc.tensor.matmul(
                                ps2[:, nt * TS:(nt + 1) * TS],
                                aT[c, gi][:, (tt % 4) * 128:(tt % 4 + 1) * 128],
                                wp[:, c * D + nt * TS:c * D + (nt + 1) * TS],
                                start=(c == 0), stop=(c == 1))
                    nc.vector.tensor_copy(o[:], ps2[:])
                else:
                    for nt in range(2):
                        ps = ps_mm.tile([128, TS], F32, tag="mm",
                                        name=f"pj_{tt}_{nt}")
                        for c in range(2):
                            nc.tensor.matmul(
                                ps[:],
                                aT[c, gi][:, (tt % 4) * 128:(tt % 4 + 1) * 128],
                                wp[:, c * D + nt * TS:c * D + (nt + 1) * TS],
                                start=(c == 0), stop=(c == 1))
                        nc.vector.tensor_copy(o[:, nt * TS:(nt + 1) * TS],
                                              ps[:])
                nc.sync.dma_start(out=out_dram[tt], in_=o[:])
            return unit

        def phase_a_units(ts):
            us = []
            if ts > 0:
                us.append(load_xr(ts))
            # all q/k groups first: they gate the next slice's attention
            for ct in (0, 2, 1, 3):
                us.append(qk_group(ts, ct))
            for sub in range(4):
                us.append(v_group(ts, sub))
            return us

        def boost(u):
            def unit():
                with tc.high_priority():
                    u()
            return unit

        def attention_units(gi, nboost=0):
            """Flat pipeline over (h, pair): av trails its exp by `depth`
            slots so the ACT stream never drains. The first `nboost` sc/exp
            units are emitted at top priority so they preempt leftover
            phase-A filler the moment their inputs are ready."""
            npairs = 2 * (gi + 1)
            seq = [(h, p) for h in range(HG) for p in range(npairs)]
            depth = min(3, npairs)
            us = []
            for idx, (h, p) in enumerate(seq):
                sc_u, ex_u = att_sc(gi, h, p), att_exp(gi, h, p)
                if idx < nboost:
                    sc_u, ex_u = boost(sc_u), boost(ex_u)
                us.append(sc_u)
                us.append(ex_u)
                if idx >= depth:
                    ph, pp = seq[idx - depth]
                    us.append(att_av(gi, ph, pp))
                    if pp == npairs - 1:
                        us.append(att_norm(gi, ph))
            for idx in range(len(seq) - depth, len(seq)):
                ph, pp = seq[idx]
                us.append(att_av(gi, ph, pp))
                if pp == npairs - 1:
                    us.append(att_norm(gi, ph))
            return us

        # slice 0: only the ct 0/2 projections must precede attention(0,h0).
        # The rest of phase A is injected into attention(0) at positions that
        # keep every producer emitted before its consumer (tile dependency
        # tracking is emission-order based): v_group(sub) before the first
        # att_av reading that j-block.
        qk_group(0, 0)()
        qk_group(0, 2)()

        def attention0_units():
            a = attention_units(0)
            # flat layout: [sc00,e00, sc01,e01, sc10,e10, av00, sc11,e11,
            #               av01, norm0, ...]
            inject = [(2, qk_group(0, 1)), (4, qk_group(0, 3)),
                      (6, v_group(0, 0)), (6, v_group(0, 1)),
                      (9, v_group(0, 2)), (9, v_group(0, 3))]
            for pos, u in reversed(inject):
                a.insert(pos, u)
            return a

        for gi in range(NTS):
            if gi < NTS - 1:
                filler = phase_a_units(gi + 1)
            else:
                filler = [proj_unit(tt) for tt in range(0, 12)]
            prim = attention0_units() if gi == 0 else attention_units(gi, nboost=2)
            for u in interleave(prim, filler, back=0.0):
                u()
        for tt in range(12, 16):
            proj_unit(tt, tail=True)()

    nc.finalize()
    return nc


_NC = None


def _get_nc():
    global _NC
    if _NC is None:
        _NC = build()
    return _NC


def _make_in_maps(x, W_attn, b_attn, W_proj):
    jj = np.arange(128, dtype=np.int64)[:, None]
    ii = np.arange(128, dtype=np.int64)[None, :]
    tri = (jj <= ii).astype(np.float32)
    mask = np.ascontiguousarray(
        np.concatenate([np.zeros((128, 128), np.float32), tri], axis=1))

    shards = []
    for g in range(4):
        q_cols = W_attn[:, g * CV:(g + 1) * CV]
        k_cols = W_attn[:, D + g * CV:D + (g + 1) * CV]
        wqk = np.concatenate([q_cols, k_cols], axis=1)          # [D, 512]
        w1 = (wqk * SW).astype(NP8)
        w2 = (wqk * SW - np.asarray(w1, np.float32)).astype(NP8)
        # [128, (k, resid, c)]
        wqk8 = np.stack([w1.reshape(KT, 128, CQK),
                         w2.reshape(KT, 128, CQK)], axis=2)     # [KT,128,2,512]
        wqk8 = np.ascontiguousarray(
            wqk8.transpose(1, 0, 2, 3).reshape(128, KT * 2 * CQK))
        wvg = np.ascontiguousarray(
            W_attn[:, 2 * D + g * CV:2 * D + (g + 1) * CV].reshape(
                KT, 128, CV).transpose(1, 0, 2).reshape(128, KT * CV)
        ).astype(NPBF)
        wpg = np.ascontiguousarray(
            W_proj[g * CV:(g + 1) * CV, :].reshape(2, 128, D).transpose(
                1, 0, 2).reshape(128, 2 * D))
        bq = b_attn[g * CV:(g + 1) * CV]
        bk = b_attn[D + g * CV:D + (g + 1) * CV]
        bqk = np.ascontiguousarray(
            np.concatenate([bq, bk]).reshape(4, 128).T).astype(np.float32)
        shards.append((wqk8, wvg, wpg, bqk))

    in_maps = []
    for b in range(B):
        xt = x[b].T.reshape(KT, 128, NTS, TS)                   # [k, p, ts, t]
        xT = np.ascontiguousarray(
            xt.transpose(2, 1, 0, 3).reshape(NTS, 128, KT * TS))
        xT8 = xT.astype(NP8)
        xTb = xT.astype(NPBF)
        for g in range(4):
            wqk8, wvg, wpg, bqk = shards[g]
            in_maps.append({
                "xT": xTb, "xT8": xT8, "wqk": wqk8, "wv": wvg, "wp": wpg,
                "bqk": bqk, "mask": mask,
            })
    return in_maps


def run(inputs, trace=False):
    x = np.asarray(inputs["x"], dtype=np.float32)
    W_attn = np.asarray(inputs["W_attn"], dtype=np.float32)
    b_attn = np.asarray(inputs["b_attn"], dtype=np.float32)
    W_proj = np.asarray(inputs["W_proj"], dtype=np.float32)
    b_proj = np.asarray(inputs["b_proj"], dtype=np.float32)

    nc = _get_nc()
    in_maps = _make_in_maps(x, W_attn, b_attn, W_proj)
    res = run_bass_kernel_spmd(nc, in_maps, list(range(8)), trace=trace)

    out = np.zeros((B, T, D), dtype=np.float32)
    for b in range(B):
        for g in range(4):
            out[b] += res.results[b * 4 + g]["out"].reshape(T, D).astype(np.float32)
    # v-bias contributes a constant shift through the value path; b_proj too.
    const = b_attn[2 * D:3 * D] @ W_proj + b_proj
    out += const[None, None, :].astype(np.float32)
    return out, res


def kernel(**inputs):
    out, _ = run(inputs, trace=False)
    return out


# revision 24
# speedup vs baseline: 1.0118x; 1.0089x over previous
"""Causal multi-head attention block (GPT-style) on 8 TRN2 NeuronCores.

Sharding: core (b, g) = batch b in {0,1} x head-group g in {0..3} (4 heads of
dh=64 each). Megatron-style: each core computes q/k/v projections for its 256
channels, attention for its 4 heads, and a partial c_proj using its 256 rows of
W_proj. Host sums the 4 partial projections per batch (+ bias terms).

fp8(e4m3) strategy, validated against the 2e-2 absmax gate (measured ~1.5e-2):
  - qk projection: DoubleRow matmul; stationary planes = (W1, W2) residual
    pair at scale 32 (W ~ (W1+W2)/32); moving planes = (x8, x8) via stride-0
    broadcast_to. 2x over f32r.
  - scores: DoubleRow zero-plane form; plane 1 of q8/k8 tiles is a zeroed
    upper half, so each instr computes k8^T q8 at 0.5 cyc/col. 2x.
  - exp on ACT reads the score psum, applies the 1/sqrt(dh)=0.125 scale inside
    the activation, and writes u directly as fp8.
  - av: DoubleRow; stationary planes = (v1, v2) fp8 residual pair of the
    bf16-projected v (plain fp8 v fails the gate); moving planes = (u, u) via
    stride-0 broadcast_to. 2x.
  - v projection runs in bf16 (x and W_v loaded as bf16 - halves the largest
    DMA transfers); c_proj stays f32r.
DMAs are batched into ~30 large transfers: the serial HWDGE device charges
625ns per DMA and SP.SEQ ~650ns, which capped the old 116-DMA schedule at
~131us regardless of engine utilization.
Engine placement: ACT does only exp (the bottleneck); DVE does the psum->fp8
casts, softmax normalization, and c_proj psum evacuation; Pool (no PSUM
access) does the causal triangle masks on u and the partition broadcasts.
"""

import sys

try:
    import concourse  # noqa: F401
except ImportError:
    sys.path.insert(0, "/opt/trn_rl_repo")

from contextlib import ExitStack

import numpy as np
import ml_dtypes

import concourse.tile as tile
from concourse import bacc, mybir
from concourse.bass_utils import run_bass_kernel_spmd

F32 = mybir.dt.float32
F32R = mybir.dt.float32r
BF16 = mybir.dt.bfloat16
F8 = mybir.dt.float8e4
F16 = mybir.dt.float16
NP8 = ml_dtypes.float8_e4m3
NPBF = ml_dtypes.bfloat16
EXP = mybir.ActivationFunctionType.Exp
MUL = mybir.AluOpType.mult
ADD = mybir.AluOpType.add
SUB = mybir.AluOpType.subtract
DR = mybir.MatmulPerfMode.DoubleRow

B, T, D = 2, 2048, 1024
HG, DH = 4, 64          # heads per core, head dim
CQK = 512               # q+k channels per core
CV = 256                # v channels per core
KT = D // 128           # contraction tiles of the projections
KA, KB = 2, 6           # k-tile split of the x/w loads (startup pipelining)
TS = 512                # t-slice width
NTS = T // TS
NT128 = T // 128
SW = 32.0               # fp8 scale of the W_qk residual pair
# Vt per j-block: [resid (2, stride 320), head (4) x (64 v + 1 ones)].
# The DoubleRow ldweights plane stride must be a multiple of 64, so the
# resid-plane stride is padded 260 -> 320.
VS = 320                # resid-plane stride inside a j-block
VB = 2 * VS             # 640 bytes per j-block in Vt


def interleave(primary, filler, back=0.45):
    """Merge filler among primary units, weighted toward the back where the
    ACT pipeline debt is largest."""
    if not filler:
        return list(primary)
    n = len(primary)
    out = []
    fi = 0
    for i, p in enumerate(primary):
        out.append(p)
        x = (i + 1) / n
        want = ((1 - back) * x + back * x * x) * len(filler)
        while fi < len(filler) and fi + 1 <= want:
            out.append(filler[fi])
            fi += 1
    out.extend(filler[fi:])
    return out


def build():
    nc = bacc.Bacc(None)

    xT_in = nc.dram_tensor("xT", [NTS, 128, KT * TS], BF16, kind="ExternalInput")
    xT8_in = nc.dram_tensor("xT8", [NTS, 128, KT * TS], F8, kind="ExternalInput")
    wqk_in = nc.dram_tensor("wqk", [2, 128, KT * 4 * 128], F8, kind="ExternalInput")
    wv_in = nc.dram_tensor("wv", [128, KT * CV], BF16, kind="ExternalInput")
    wp_in = nc.dram_tensor("wp", [128, 2 * D], F32R, kind="ExternalInput")
    bias_in = nc.dram_tensor("bqk", [128, 4], F32, kind="ExternalInput")
    mask_in = nc.dram_tensor("mask", [128, 256], F32R, kind="ExternalInput")
    out_dram = nc.dram_tensor("out", [NT128, 128, 2 * TS], F16, kind="ExternalOutput")

    with ExitStack() as ctx:
        tc = ctx.enter_context(tile.TileContext(nc))

        const = ctx.enter_context(tc.tile_pool(name="const", bufs=1))
        big = ctx.enter_context(tc.tile_pool(name="big", bufs=1))
        upool = ctx.enter_context(tc.tile_pool(name="upool", bufs=8))
        rows = ctx.enter_context(tc.tile_pool(name="rows", bufs=2))
        rbcp = ctx.enter_context(tc.tile_pool(name="rbcp", bufs=3))
        outp = ctx.enter_context(tc.tile_pool(name="outp", bufs=4))
        xrp = ctx.enter_context(tc.tile_pool(name="xrp", bufs=2))
        xr8p = ctx.enter_context(tc.tile_pool(name="xr8p", bufs=2))

        ps_s = ctx.enter_context(tc.tile_pool(name="ps_s", bufs=2, space="PSUM"))
        ps_av = ctx.enter_context(tc.tile_pool(name="ps_av", bufs=2, space="PSUM"))
        ps_mm = ctx.enter_context(tc.tile_pool(name="ps_mm", bufs=2, space="PSUM"))

        # weight + first-slice DMAs, startup-critical first: the first two
        # qk projections (ct 0 and 2) need the ct{0,2} weight group + x8(0).
        wqk02 = const.tile([128, KT * 4 * 128], F8, tag="wqk02")
        nc.sync.dma_start(out=wqk02[:], in_=wqk_in[0])
        xr, xr8 = {}, {}

        def load_x8(ts):
            x8 = xr8p.tile([128, KT * TS], F8, tag="xr8", name=f"xr8_{ts}")
            nc.sync.dma_start(out=x8[:], in_=xT8_in[ts])
            xr8[ts] = x8

        def load_xbf(ts):
            x1 = xrp.tile([128, KT * TS], BF16, tag="xra", name=f"xra_{ts}")
            nc.sync.dma_start(out=x1[:], in_=xT_in[ts])
            xr[ts] = x1

        def load_xr(ts):
            def unit():
                load_x8(ts)
                load_xbf(ts)
            return unit

        load_x8(0)
        wqk13 = const.tile([128, KT * 4 * 128], F8, tag="wqk13")
        nc.sync.dma_start(out=wqk13[:], in_=wqk_in[1])
        bias_sb = const.tile([128, 4], F32, tag="bias")
        nc.sync.dma_start(out=bias_sb[:], in_=bias_in[:])
        wv = const.tile([128, KT * CV], BF16, tag="wv")
        nc.sync.dma_start(out=wv[:], in_=wv_in[:])
        tri = const.tile([128, 256], F32R, tag="tri")  # [zeros(128) | tri(128)]
        nc.sync.dma_start(out=tri[:], in_=mask_in[:])
        load_xbf(0)
        wp = const.tile([128, 2 * D], F32R, tag="wp")
        nc.sync.dma_start(out=wp[:], in_=wp_in[:])

        def xr8_slice(ts, k):
            return xr8[ts][:, k * TS:(k + 1) * TS]

        def xr_slice(ts, k, lo, hi):
            return xr[ts][:, k * TS + lo:k * TS + hi]

        import dataclasses

        def _plane(t, off, stride=CQK, m=128, p0=0, pk=128):
            """Stationary DoubleRow AP [pk, 2(stride), m] at free offset off.
            Built from a 1-element slice so tile dep-tracking sees tile t;
            the stride must be a multiple of 64 (dual-fp8 ldweights rule)."""
            ap = t[p0:p0 + pk, off:off + 1].unsqueeze(1)
            return dataclasses.replace(
                ap, ap=[ap.ap[0], (stride, 2), (1, m)])

        ones128 = const.tile([128, HG], F32, tag="ones128")
        nc.vector.memset(ones128[:], 1.0)

        # persistent intermediates
        # q8/k8 per (ct, ts): [128, 1024] fp8; [0:512) data, [512:1024) zeros
        # (the zeros feed the unused DoubleRow plane). ct 0/1 = q of head-pair
        # 0/1, ct 2/3 = k of head-pair 0/1.
        qkT = {(ct, ts): big.tile([128, 2 * TS], F8, tag=f"qkT{ct}_{ts}",
                                  name=f"qkT{ct}_{ts}")
               for ct in range(4) for ts in range(NTS)}
        Vt = [big.tile([128, 4 * VB], F8, tag=f"Vt{ts}", name=f"Vt{ts}")
              for ts in range(NTS)]
        aT = {(hp, gi): big.tile([128, TS], F32R, tag=f"aT{hp}_{gi}",
                                 name=f"aT{hp}_{gi}")
              for hp in range(2) for gi in range(NTS)}

        # zero the DoubleRow upper halves once; set the v ones/zeros columns.
        # Order matters: slice-0 consumers run first, so init ts=0 tiles
        # first (qkT(0,0) fully - it doubles as the PE warmup operand).
        def init_vt(ts):
            v5 = Vt[ts][:].rearrange("p (s r q) -> p s r q", s=4, r=2)
            for r in range(2):
                ve = v5[:, :, r, 0:HG * (DH + 1)].rearrange(
                    "p s (h e) -> p s h e", e=DH + 1)
                if r == 0:
                    nc.gpsimd.tensor_copy(
                        ve[:, :, :, DH],
                        ones128[:].unsqueeze(1).broadcast_to((128, 4, HG)))
                else:
                    nc.gpsimd.memset(ve[:, :, :, DH], 0.0)

        nc.gpsimd.memset(qkT[0, 0][:], 0.0)
        for ct in (2, 1, 3):
            nc.gpsimd.memset(qkT[ct, 0][:, TS:], 0.0)
        init_vt(0)

        # PE warmup: the tensor engine runs at reduced clock for its first
        # ~3us of execution. Burn the p-state ramp on zero matmuls while the
        # first x8/wqk DMAs are still in flight.
        wps = ps_s.tile([128, 2 * TS], F32, tag="ss", name="warm")
        wmov = qkT[0, 0][:, 0:TS].unsqueeze(1).broadcast_to((128, 2, TS))
        for i in range(7):
            nc.tensor.matmul(wps[:, 0:TS], _plane(qkT[0, 0], 0, stride=TS),
                             wmov, start=True, stop=True, perf_mode=DR)

        for ts in range(1, NTS):
            for ct in range(4):
                nc.gpsimd.memset(qkT[ct, ts][:, TS:], 0.0)
            init_vt(ts)

        def qk_group(ts, ct):
            """DoubleRow projection of 128 q/k channels for slice ts:
            psum = x8 @ (W1+W2) = 32*qk; DVE casts to fp8 with 1/32 (+bias)."""
            def unit():
                ps = ps_mm.tile([128, TS], F32, tag="mm", name=f"qk_{ts}_{ct}")
                t = wqk02 if ct in (0, 2) else wqk13
                for k in range(KT):
                    wst = _plane(t, k * 512 + (ct // 2) * 128, stride=256)
                    xb = xr8_slice(ts, k).unsqueeze(1).broadcast_to((128, 2, TS))
                    nc.tensor.matmul(ps[:], wst, xb,
                                     start=(k == 0), stop=(k == KT - 1),
                                     perf_mode=DR)
                nc.vector.tensor_scalar(
                    qkT[ct, ts][:, 0:TS], ps[:],
                    1.0 / SW, bias_sb[:, ct:ct + 1], op0=MUL, op1=ADD)
            return unit

        def v_group(ts, sub):
            """bf16 v projection of j-block sub of slice ts, then fp8
            residual-pair cast into Vt[ts]."""
            def unit():
                ps = ps_mm.tile([128, CV], F32, tag="mm", name=f"v_{ts}_{sub}")
                for k in range(KT):
                    nc.tensor.matmul(ps[:],
                                     xr_slice(ts, k, sub * 128, (sub + 1) * 128),
                                     wv[:, k * CV:(k + 1) * CV],
                                     start=(k == 0), stop=(k == KT - 1))
                v5 = Vt[ts][:].rearrange("p (s r q) -> p s r q", s=4, r=2)
                v0 = v5[:, sub, 0, 0:HG * (DH + 1)].rearrange(
                    "p (h e) -> p h e", e=DH + 1)[:, :, 0:DH]
                v1 = v5[:, sub, 1, 0:HG * (DH + 1)].rearrange(
                    "p (h e) -> p h e", e=DH + 1)[:, :, 0:DH]
                p3 = ps[:].rearrange("p (h e) -> p h e", e=DH)
                nc.vector.tensor_copy(v0, p3)
                nc.vector.tensor_tensor(v1, p3, v0, op=SUB)
            return unit

        # ---- attention ----
        sstiles = {}
        utiles = {}
        avtiles = {}

        def att_sc(gi, h, p):
            """Scores for j-tile pair (2p, 2p+1): two DoubleRow zero-plane
            matmuls into one [128, 2, TS] psum tile."""
            npairs = 2 * (gi + 1)
            c0 = 256 if p == npairs - 1 else 0
            hp, p0 = h // 2, 64 * (h % 2)

            def unit():
                ss = ps_s.tile([128, 2 * TS], F32, tag="ss",
                               name=f"ss_{gi}_{h}_{p}")
                sstiles[gi, h, p] = ss
                s3 = ss[:].rearrange("q (j i) -> q j i", j=2)
                qm = qkT[hp, gi][p0:p0 + 64, :].rearrange(
                    "q (r i) -> q r i", r=2)[:, :, c0:TS]
                for jj in range(2):
                    jt = 2 * p + jj
                    jts, jo = jt // 4, (jt % 4) * 128
                    km = _plane(qkT[2 + hp, jts], jo, stride=TS, m=128,
                                p0=p0, pk=64)
                    nc.tensor.matmul(s3[:, jj, c0:TS], km, qm,
                                     start=True, stop=True, perf_mode=DR)
            return unit

        def att_exp(gi, h, p):
            """exp of the score pair -> fp8 u tile; Pool masks the causal
            boundary on the diagonal pairs."""
            npairs = 2 * (gi + 1)
            c0 = 256 if p == npairs - 1 else 0

            def unit():
                ss = sstiles.pop((gi, h, p))
                u = upool.tile([128, 2 * TS], F8, tag="u",
                               name=f"u_{gi}_{h}_{p}")
                utiles[gi, h, p] = u
                u3 = u[:].rearrange("q (j i) -> q j i", j=2)
                s3 = ss[:].rearrange("q (j i) -> q j i", j=2)
                nc.scalar.activation(u3[:, :, c0:TS], s3[:, :, c0:TS], EXP,
                                     scale=0.125)
                # Pool is slow per-op; at the very end of the schedule its
                # latency sits on the critical path, so the last head's
                # masks run on DVE instead.
                veng = nc.vector if (gi == NTS - 1 and h == HG - 1) else nc.gpsimd
                if p == 2 * gi:        # diagonal pair (d = 0, 128)
                    veng.tensor_tensor(
                        u3[:, 0, 0:128], u3[:, 0, 0:128],
                        tri[:, 128:256], op=MUL)
                    veng.tensor_tensor(
                        u3[:, 1, 0:256], u3[:, 1, 0:256],
                        tri[:, 0:256], op=MUL)
                elif p == 2 * gi + 1:  # diagonal pair (d = 256, 384)
                    veng.tensor_tensor(
                        u3[:, 0, 256:384], u3[:, 0, 256:384],
                        tri[:, 128:256], op=MUL)
                    veng.tensor_tensor(
                        u3[:, 1, 256:512], u3[:, 1, 256:512],
                        tri[:, 0:256], op=MUL)
            return unit

        def att_av(gi, h, p):
            """attn-out accumulation for the pair: two DoubleRow matmuls with
            v-residual stationary planes and stride-0 u moving planes."""
            npairs = 2 * (gi + 1)

            def unit():
                if p == 0:
                    avtiles[gi, h] = ps_av.tile([DH + 1, TS], F32, tag="av",
                                                name=f"av_{gi}_{h}")
                av = avtiles[gi, h]
                u = utiles[gi, h, p]
                if p == npairs - 1:
                    utiles.pop((gi, h, p))
                u3 = u[:].rearrange("q (j i) -> q j i", j=2)
                for jj in range(2):
                    jt = 2 * p + jj
                    d = jt * 128 - gi * TS
                    c0 = max(d, 0)
                    jts, jb = jt // 4, jt % 4
                    vst = _plane(Vt[jts], jb * VB + h * (DH + 1),
                                 stride=VS, m=DH + 1)
                    um = u3[:, jj, c0:TS].unsqueeze(1).broadcast_to(
                        (128, 2, TS - c0))
                    nc.tensor.matmul(av[:, c0:TS], vst, um,
                                     start=(jt == 0), stop=(jt == 2 * npairs - 1),
                                     perf_mode=DR)
            return unit

        def att_norm(gi, h):
            """softmax normalization: 1/denominator broadcast-multiplied into
            aT, straight from the av psum tile."""
            hp, p0 = h // 2, 64 * (h % 2)

            def unit():
                av = avtiles.pop((gi, h))
                r = rows.tile([1, TS], F32, tag="r", name=f"r_{gi}_{h}")
                nc.vector.reciprocal(r[:], av[DH:DH + 1, :])
                rbc = rbcp.tile([DH, TS], F32, tag="rbc", name=f"rbc_{gi}_{h}")
                nc.gpsimd.partition_broadcast(rbc[:], r[:])
                nc.vector.tensor_tensor(
                    aT[hp, gi][p0:p0 + 64, :], av[0:DH, :], rbc[:], op=MUL)
            return unit

        def proj_unit(tt, tail=False, act_copy=False):
            gi = tt // 4

            def unit():
                o = outp.tile([128, 2 * TS], F16, tag="o", name=f"o_{tt}")
                if tail:
                    # attention psum pools are free by now; one 2-bank tile
                    # holds both nt accumulation groups -> single merged copy
                    ps2 = ps_s.tile([128, 2 * TS], F32, tag="ss",
                                    name=f"pj_{tt}")
                    for nt in range(2):
                        for c in range(2):
                            nc.tensor.matmul(
                                ps2[:, nt * TS:(nt + 1) * TS],
                                aT[c, gi][:, (tt % 4) * 128:(tt % 4 + 1) * 128],
                                wp[:, c * D + nt * TS:c * D + (nt + 1) * TS],
                                start=(c == 0), stop=(c == 1))
                    if act_copy:
                        nc.scalar.copy(o[:], ps2[:])
                    else:
                        nc.vector.tensor_copy(o[:], ps2[:])
                else:
                    for nt in range(2):
                        ps = ps_mm.tile([128, TS], F32, tag="mm",
                                        name=f"pj_{tt}_{nt}")
                        for c in range(2):
                            nc.tensor.matmul(
                                ps[:],
                                aT[c, gi][:, (tt % 4) * 128:(tt % 4 + 1) * 128],
                                wp[:, c * D + nt * TS:c * D + (nt + 1) * TS],
                                start=(c == 0), stop=(c == 1))
                        nc.vector.tensor_copy(o[:, nt * TS:(nt + 1) * TS],
                                              ps[:])
                nc.sync.dma_start(out=out_dram[tt], in_=o[:])
            return unit

        def phase_a_units(ts):
            us = []
            if ts > 0:
                us.append(load_xr(ts))
            # all q/k groups first: they gate the next slice's attention
            for ct in (0, 2, 1, 3):
                us.append(qk_group(ts, ct))
            for sub in range(4):
                us.append(v_group(ts, sub))
            return us

        def boost(u):
            def unit():
                with tc.high_priority():
                    u()
            return unit

        def attention_units(gi, nboost=0):
            """Flat pipeline over (h, pair): av trails its exp by `depth`
            slots so the ACT stream never drains. The first `nboost` sc/exp
            units are emitted at top priority so they preempt leftover
            phase-A filler the moment their inputs are ready."""
            npairs = 2 * (gi + 1)
            seq = [(h, p) for h in range(HG) for p in range(npairs)]
            depth = min(3, npairs)
            us = []
            for idx, (h, p) in enumerate(seq):
                sc_u, ex_u = att_sc(gi, h, p), att_exp(gi, h, p)
                if idx < nboost:
                    sc_u, ex_u = boost(sc_u), boost(ex_u)
                us.append(sc_u)
                us.append(ex_u)
                if idx >= depth:
                    ph, pp = seq[idx - depth]
                    us.append(att_av(gi, ph, pp))
                    if pp == npairs - 1:
                        us.append(att_norm(gi, ph))
            for idx in range(len(seq) - depth, len(seq)):
                ph, pp = seq[idx]
                us.append(att_av(gi, ph, pp))
                if pp == npairs - 1:
                    us.append(att_norm(gi, ph))
            return us

        # slice 0: only the ct 0/2 projections must precede attention(0,h0).
        # The rest of phase A is injected into attention(0) at positions that
        # keep every producer emitted before its consumer (tile dependency
        # tracking is emission-order based): v_group(sub) before the first
        # att_av reading that j-block.
        qk_group(0, 0)()
        qk_group(0, 2)()

        def attention0_units():
            a = attention_units(0)
            # flat layout: [sc00,e00, sc01,e01, sc10,e10, av00, sc11,e11,
            #               av01, norm0, ...]
            inject = [(2, qk_group(0, 1)), (4, qk_group(0, 3)),
                      (6, v_group(0, 0)), (6, v_group(0, 1)),
                      (9, v_group(0, 2)), (9, v_group(0, 3))]
            for pos, u in reversed(inject):
                a.insert(pos, u)
            return a

        for gi in range(NTS):
            if gi < NTS - 1:
                filler = phase_a_units(gi + 1)
            else:
                filler = [proj_unit(tt) for tt in range(0, 12)]
            prim = attention0_units() if gi == 0 else attention_units(gi, nboost=2)
            for u in interleave(prim, filler, back=0.0):
                u()
        for tt in range(12, 16):
            proj_unit(tt, tail=True, act_copy=(tt % 2 == 1))()

    nc.finalize()
    return nc


_NC = None


def _get_nc():
    global _NC
    if _NC is None:
        _NC = build()
    return _NC


def _make_in_maps(x, W_attn, b_attn, W_proj):
    jj = np.arange(128, dtype=np.int64)[:, None]
    ii = np.arange(128, dtype=np.int64)[None, :]
    tri = (jj <= ii).astype(np.float32)
    mask = np.ascontiguousarray(
        np.concatenate([np.zeros((128, 128), np.float32), tri], axis=1))

    shards = []
    for g in range(4):
        q_cols = W_attn[:, g * CV:(g + 1) * CV]
        k_cols = W_attn[:, D + g * CV:D + (g + 1) * CV]
        wqk = np.concatenate([q_cols, k_cols], axis=1)          # [D, 512]
        w1 = (wqk * SW).astype(NP8)
        w2 = (wqk * SW - np.asarray(w1, np.float32)).astype(NP8)
        # [ctpair, 128, (k, resid, ctl, c)]: ct groups {0,2} and {1,3}
        wr = np.stack([w1.reshape(KT, 128, 4, 128),
                       w2.reshape(KT, 128, 4, 128)], axis=1)    # [KT,2r,128,4ct,128]
        wqk8 = np.ascontiguousarray(np.stack(
            [wr[:, :, :, (0, 2), :], wr[:, :, :, (1, 3), :]], axis=0
        ).transpose(0, 3, 1, 2, 4, 5).reshape(2, 128, KT * 4 * 128))
        wvg = np.ascontiguousarray(
            W_attn[:, 2 * D + g * CV:2 * D + (g + 1) * CV].reshape(
                KT, 128, CV).transpose(1, 0, 2).reshape(128, KT * CV)
        ).astype(NPBF)
        wpg = np.ascontiguousarray(
            W_proj[g * CV:(g + 1) * CV, :].reshape(2, 128, D).transpose(
                1, 0, 2).reshape(128, 2 * D))
        bq = b_attn[g * CV:(g + 1) * CV]
        bk = b_attn[D + g * CV:D + (g + 1) * CV]
        bqk = np.ascontiguousarray(
            np.concatenate([bq, bk]).reshape(4, 128).T).astype(np.float32)
        shards.append((wqk8, wvg, wpg, bqk))

    in_maps = []
    for b in range(B):
        xt = x[b].T.reshape(KT, 128, NTS, TS)                   # [k, p, ts, t]
        xT = np.ascontiguousarray(
            xt.transpose(2, 1, 0, 3).reshape(NTS, 128, KT * TS))
        xT8 = xT.astype(NP8)
        xTb = xT.astype(NPBF)
        for g in range(4):
            wqk8, wvg, wpg, bqk = shards[g]
            in_maps.append({
                "xT": xTb, "xT8": xT8, "wqk": wqk8, "wv": wvg, "wp": wpg,
                "bqk": bqk, "mask": mask,
            })
    return in_maps


def run(inputs, trace=False):
    x = np.asarray(inputs["x"], dtype=np.float32)
    W_attn = np.asarray(inputs["W_attn"], dtype=np.float32)
    b_attn = np.asarray(inputs["b_attn"], dtype=np.float32)
    W_proj = np.asarray(inputs["W_proj"], dtype=np.float32)
    b_proj = np.asarray(inputs["b_proj"], dtype=np.float32)

    nc = _get_nc()
    in_maps = _make_in_maps(x, W_attn, b_attn, W_proj)
    res = run_bass_kernel_spmd(nc, in_maps, list(range(8)), trace=trace)

    out = np.zeros((B, T, D), dtype=np.float32)
    for b in range(B):
        for g in range(4):
            out[b] += res.results[b * 4 + g]["out"].reshape(T, D).astype(np.float32)
    # v-bias contributes a constant shift through the value path; b_proj too.
    const = b_attn[2 * D:3 * D] @ W_proj + b_proj
    out += const[None, None, :].astype(np.float32)
    return out, res


def kernel(**inputs):
    out, _ = run(inputs, trace=False)
    return out


# revision 25
# speedup vs baseline: 1.0378x; 1.0257x over previous
"""Causal multi-head attention block (GPT-style) on 8 TRN2 NeuronCores.

Sharding: core (b, g) = batch b in {0,1} x head-group g in {0..3} (4 heads of
dh=64 each). Megatron-style: each core computes q/k/v projections for its 256
channels, attention for its 4 heads, and a partial c_proj using its 256 rows of
W_proj. Host sums the 4 partial projections per batch (+ bias terms).

fp8(e4m3) strategy, validated against the 2e-2 absmax gate (measured ~1.5e-2):
  - qk projection: DoubleRow matmul; stationary planes = (W1, W2) residual
    pair at scale 32 (W ~ (W1+W2)/32); moving planes = (x8, x8) via stride-0
    broadcast_to. 2x over f32r.
  - scores: DoubleRow zero-plane form; plane 1 of q8/k8 tiles is a zeroed
    upper half, so each instr computes k8^T q8 at 0.5 cyc/col. 2x.
  - exp on ACT reads the score psum, applies the 1/sqrt(dh)=0.125 scale inside
    the activation, and writes u directly as fp8.
  - av: DoubleRow; stationary planes = (v1, v2) fp8 residual pair of the
    bf16-projected v (plain fp8 v fails the gate); moving planes = (u, u) via
    stride-0 broadcast_to. 2x.
  - v projection runs in bf16 (x and W_v loaded as bf16 - halves the largest
    DMA transfers); c_proj stays f32r.
DMAs are batched into ~30 large transfers: the serial HWDGE device charges
625ns per DMA and SP.SEQ ~650ns, which capped the old 116-DMA schedule at
~131us regardless of engine utilization.
Engine placement: ACT does only exp (the bottleneck); DVE does the psum->fp8
casts, softmax normalization, and c_proj psum evacuation; Pool (no PSUM
access) does the causal triangle masks on u and the partition broadcasts.
"""

import sys

try:
    import concourse  # noqa: F401
except ImportError:
    sys.path.insert(0, "/opt/trn_rl_repo")

from contextlib import ExitStack

import numpy as np
import ml_dtypes

import concourse.tile as tile
from concourse import bacc, mybir
from concourse.bass_utils import run_bass_kernel_spmd

F32 = mybir.dt.float32
F32R = mybir.dt.float32r
BF16 = mybir.dt.bfloat16
F8 = mybir.dt.float8e4
F16 = mybir.dt.float16
NP8 = ml_dtypes.float8_e4m3
NPBF = ml_dtypes.bfloat16
EXP = mybir.ActivationFunctionType.Exp
MUL = mybir.AluOpType.mult
ADD = mybir.AluOpType.add
SUB = mybir.AluOpType.subtract
DR = mybir.MatmulPerfMode.DoubleRow

B, T, D = 2, 2048, 1024
HG, DH = 4, 64          # heads per core, head dim
CQK = 512               # q+k channels per core
CV = 256                # v channels per core
KT = D // 128           # contraction tiles of the projections
KA, KB = 2, 6           # k-tile split of the x/w loads (startup pipelining)
TS = 512                # t-slice width
NTS = T // TS
NT128 = T // 128
SW = 32.0               # fp8 scale of the W_qk residual pair
# Vt per j-block: [resid (2, stride 320), head (4) x (64 v + 1 ones)].
# The DoubleRow ldweights plane stride must be a multiple of 64, so the
# resid-plane stride is padded 260 -> 320.
VS = 320                # resid-plane stride inside a j-block
VB = 2 * VS             # 640 bytes per j-block in Vt


def interleave(primary, filler, back=0.45):
    """Merge filler among primary units, weighted toward the back where the
    ACT pipeline debt is largest."""
    if not filler:
        return list(primary)
    n = len(primary)
    out = []
    fi = 0
    for i, p in enumerate(primary):
        out.append(p)
        x = (i + 1) / n
        want = ((1 - back) * x + back * x * x) * len(filler)
        while fi < len(filler) and fi + 1 <= want:
            out.append(filler[fi])
            fi += 1
    out.extend(filler[fi:])
    return out


def build():
    nc = bacc.Bacc(None)

    xT_in = nc.dram_tensor("xT", [NTS, 128, KT * TS], BF16, kind="ExternalInput")
    xT8_in = nc.dram_tensor("xT8", [NTS, 128, KT * TS], F8, kind="ExternalInput")
    wqk_in = nc.dram_tensor("wqk", [2, 128, KT * 4 * 128], F8, kind="ExternalInput")
    wv_in = nc.dram_tensor("wv", [128, KT * CV], BF16, kind="ExternalInput")
    wp_in = nc.dram_tensor("wp", [128, 2 * D], F32R, kind="ExternalInput")
    bias_in = nc.dram_tensor("bqk", [128, 4], F32, kind="ExternalInput")
    mask_in = nc.dram_tensor("mask", [128, 256], F32R, kind="ExternalInput")
    out_dram = nc.dram_tensor("out", [NT128, 128, 2 * TS], F16, kind="ExternalOutput")

    with ExitStack() as ctx:
        tc = ctx.enter_context(tile.TileContext(nc))

        const = ctx.enter_context(tc.tile_pool(name="const", bufs=1))
        big = ctx.enter_context(tc.tile_pool(name="big", bufs=1))
        upool = ctx.enter_context(tc.tile_pool(name="upool", bufs=8))
        rows = ctx.enter_context(tc.tile_pool(name="rows", bufs=2))
        rbcp = ctx.enter_context(tc.tile_pool(name="rbcp", bufs=3))
        outp = ctx.enter_context(tc.tile_pool(name="outp", bufs=4))
        xrp = ctx.enter_context(tc.tile_pool(name="xrp", bufs=2))
        xr8p = ctx.enter_context(tc.tile_pool(name="xr8p", bufs=2))

        ps_s = ctx.enter_context(tc.tile_pool(name="ps_s", bufs=2, space="PSUM"))
        ps_av = ctx.enter_context(tc.tile_pool(name="ps_av", bufs=2, space="PSUM"))
        ps_mm = ctx.enter_context(tc.tile_pool(name="ps_mm", bufs=2, space="PSUM"))

        # weight + first-slice DMAs, startup-critical first: the first two
        # qk projections (ct 0 and 2) need the ct{0,2} weight group + x8(0).
        wqk02 = const.tile([128, KT * 4 * 128], F8, tag="wqk02")
        nc.sync.dma_start(out=wqk02[:], in_=wqk_in[0])
        xr, xr8 = {}, {}

        def load_x8(ts):
            x8 = xr8p.tile([128, KT * TS], F8, tag="xr8", name=f"xr8_{ts}")
            nc.sync.dma_start(out=x8[:], in_=xT8_in[ts])
            xr8[ts] = x8

        def load_xbf(ts):
            x1 = xrp.tile([128, KT * TS], BF16, tag="xra", name=f"xra_{ts}")
            nc.sync.dma_start(out=x1[:], in_=xT_in[ts])
            xr[ts] = x1

        def load_xr(ts):
            def unit():
                load_x8(ts)
                load_xbf(ts)
            return unit

        load_x8(0)
        wqk13 = const.tile([128, KT * 4 * 128], F8, tag="wqk13")
        nc.sync.dma_start(out=wqk13[:], in_=wqk_in[1])
        bias_sb = const.tile([128, 4], F32, tag="bias")
        nc.sync.dma_start(out=bias_sb[:], in_=bias_in[:])
        wv = const.tile([128, KT * CV], BF16, tag="wv")
        nc.sync.dma_start(out=wv[:], in_=wv_in[:])
        tri = const.tile([128, 256], F32R, tag="tri")  # [zeros(128) | tri(128)]
        nc.sync.dma_start(out=tri[:], in_=mask_in[:])
        load_xbf(0)
        wp = const.tile([128, 2 * D], F32R, tag="wp")
        nc.sync.dma_start(out=wp[:], in_=wp_in[:])

        def xr8_slice(ts, k):
            return xr8[ts][:, k * TS:(k + 1) * TS]

        def xr_slice(ts, k, lo, hi):
            return xr[ts][:, k * TS + lo:k * TS + hi]

        import dataclasses

        def _plane(t, off, stride=CQK, m=128, p0=0, pk=128):
            """Stationary DoubleRow AP [pk, 2(stride), m] at free offset off.
            Built from a 1-element slice so tile dep-tracking sees tile t;
            the stride must be a multiple of 64 (dual-fp8 ldweights rule)."""
            ap = t[p0:p0 + pk, off:off + 1].unsqueeze(1)
            return dataclasses.replace(
                ap, ap=[ap.ap[0], (stride, 2), (1, m)])

        ones128 = const.tile([128, HG], F32, tag="ones128")
        nc.vector.memset(ones128[:], 1.0)

        # persistent intermediates
        # q8/k8 per (ct, ts): [128, 1024] fp8; [0:512) data, [512:1024) zeros
        # (the zeros feed the unused DoubleRow plane). ct 0/1 = q of head-pair
        # 0/1, ct 2/3 = k of head-pair 0/1.
        qkT = {(ct, ts): big.tile([128, 2 * TS], F8, tag=f"qkT{ct}_{ts}",
                                  name=f"qkT{ct}_{ts}")
               for ct in range(4) for ts in range(NTS)}
        Vt = [big.tile([128, 4 * VB], F8, tag=f"Vt{ts}", name=f"Vt{ts}")
              for ts in range(NTS)]
        aT = {(hp, gi): big.tile([128, TS], F32R, tag=f"aT{hp}_{gi}",
                                 name=f"aT{hp}_{gi}")
              for hp in range(2) for gi in range(NTS)}

        # zero the DoubleRow upper halves once; set the v ones/zeros columns.
        # Order matters: slice-0 consumers run first, so init ts=0 tiles
        # first (qkT(0,0) fully - it doubles as the PE warmup operand).
        def init_vt(ts):
            v5 = Vt[ts][:].rearrange("p (s r q) -> p s r q", s=4, r=2)
            for r in range(2):
                ve = v5[:, :, r, 0:HG * (DH + 1)].rearrange(
                    "p s (h e) -> p s h e", e=DH + 1)
                if r == 0:
                    nc.gpsimd.tensor_copy(
                        ve[:, :, :, DH],
                        ones128[:].unsqueeze(1).broadcast_to((128, 4, HG)))
                else:
                    nc.gpsimd.memset(ve[:, :, :, DH], 0.0)

        nc.gpsimd.memset(qkT[0, 0][:], 0.0)
        for ct in (2, 1, 3):
            nc.gpsimd.memset(qkT[ct, 0][:, TS:], 0.0)
        init_vt(0)

        # PE warmup: the tensor engine runs at reduced clock for its first
        # ~3us of execution. Burn the p-state ramp on zero matmuls while the
        # first x8/wqk DMAs are still in flight.
        wps = ps_s.tile([128, 2 * TS], F32, tag="ss", name="warm")
        wmov = qkT[0, 0][:, 0:TS].unsqueeze(1).broadcast_to((128, 2, TS))
        for i in range(7):
            nc.tensor.matmul(wps[:, 0:TS], _plane(qkT[0, 0], 0, stride=TS),
                             wmov, start=True, stop=True, perf_mode=DR)

        for ts in range(1, NTS):
            for ct in range(4):
                nc.gpsimd.memset(qkT[ct, ts][:, TS:], 0.0)
            init_vt(ts)

        def qk_group(ts, ct):
            """DoubleRow projection of 128 q/k channels for slice ts:
            psum = x8 @ (W1+W2) = 32*qk; DVE casts to fp8 with 1/32 (+bias)."""
            def unit():
                ps = ps_mm.tile([128, TS], F32, tag="mm", name=f"qk_{ts}_{ct}")
                t = wqk02 if ct in (0, 2) else wqk13
                for k in range(KT):
                    wst = _plane(t, k * 512 + (ct // 2) * 128, stride=256)
                    xb = xr8_slice(ts, k).unsqueeze(1).broadcast_to((128, 2, TS))
                    nc.tensor.matmul(ps[:], wst, xb,
                                     start=(k == 0), stop=(k == KT - 1),
                                     perf_mode=DR)
                nc.vector.tensor_scalar(
                    qkT[ct, ts][:, 0:TS], ps[:],
                    1.0 / SW, bias_sb[:, ct:ct + 1], op0=MUL, op1=ADD)
            return unit

        def v_group(ts, sub):
            """bf16 v projection of j-block sub of slice ts, then fp8
            residual-pair cast into Vt[ts]."""
            def unit():
                ps = ps_mm.tile([128, CV], F32, tag="mm", name=f"v_{ts}_{sub}")
                for k in range(KT):
                    nc.tensor.matmul(ps[:],
                                     xr_slice(ts, k, sub * 128, (sub + 1) * 128),
                                     wv[:, k * CV:(k + 1) * CV],
                                     start=(k == 0), stop=(k == KT - 1))
                v5 = Vt[ts][:].rearrange("p (s r q) -> p s r q", s=4, r=2)
                v0 = v5[:, sub, 0, 0:HG * (DH + 1)].rearrange(
                    "p (h e) -> p h e", e=DH + 1)[:, :, 0:DH]
                v1 = v5[:, sub, 1, 0:HG * (DH + 1)].rearrange(
                    "p (h e) -> p h e", e=DH + 1)[:, :, 0:DH]
                p3 = ps[:].rearrange("p (h e) -> p h e", e=DH)
                nc.vector.tensor_copy(v0, p3)
                nc.vector.tensor_tensor(v1, p3, v0, op=SUB)
            return unit

        # ---- attention ----
        sstiles = {}
        utiles = {}
        avtiles = {}

        def att_sc(gi, h, p):
            """Scores for j-tile pair (2p, 2p+1): two DoubleRow zero-plane
            matmuls into one [128, 2, TS] psum tile."""
            npairs = 2 * (gi + 1)
            c0 = 256 if p == npairs - 1 else 0
            hp, p0 = h // 2, 64 * (h % 2)

            def unit():
                ss = ps_s.tile([128, 2 * TS], F32, tag="ss",
                               name=f"ss_{gi}_{h}_{p}")
                sstiles[gi, h, p] = ss
                s3 = ss[:].rearrange("q (j i) -> q j i", j=2)
                qm = qkT[hp, gi][p0:p0 + 64, :].rearrange(
                    "q (r i) -> q r i", r=2)[:, :, c0:TS]
                for jj in range(2):
                    jt = 2 * p + jj
                    jts, jo = jt // 4, (jt % 4) * 128
                    km = _plane(qkT[2 + hp, jts], jo, stride=TS, m=128,
                                p0=p0, pk=64)
                    nc.tensor.matmul(s3[:, jj, c0:TS], km, qm,
                                     start=True, stop=True, perf_mode=DR)
            return unit

        def att_exp(gi, h, p):
            """exp of the score pair -> fp8 u tile; Pool masks the causal
            boundary on the diagonal pairs."""
            npairs = 2 * (gi + 1)
            c0 = 256 if p == npairs - 1 else 0

            def unit():
                ss = sstiles.pop((gi, h, p))
                u = upool.tile([128, 2 * TS], F8, tag="u",
                               name=f"u_{gi}_{h}_{p}")
                utiles[gi, h, p] = u
                u3 = u[:].rearrange("q (j i) -> q j i", j=2)
                s3 = ss[:].rearrange("q (j i) -> q j i", j=2)
                nc.scalar.activation(u3[:, :, c0:TS], s3[:, :, c0:TS], EXP,
                                     scale=0.125)
                # Pool is slow per-op; at the very end of the schedule its
                # latency sits on the critical path, so the last head's
                # masks run on DVE instead.
                veng = nc.vector if (gi == NTS - 1 and h == HG - 1) else nc.gpsimd
                if p == 2 * gi:        # diagonal pair (d = 0, 128)
                    veng.tensor_tensor(
                        u3[:, 0, 0:128], u3[:, 0, 0:128],
                        tri[:, 128:256], op=MUL)
                    veng.tensor_tensor(
                        u3[:, 1, 0:256], u3[:, 1, 0:256],
                        tri[:, 0:256], op=MUL)
                elif p == 2 * gi + 1:  # diagonal pair (d = 256, 384)
                    veng.tensor_tensor(
                        u3[:, 0, 256:384], u3[:, 0, 256:384],
                        tri[:, 128:256], op=MUL)
                    veng.tensor_tensor(
                        u3[:, 1, 256:512], u3[:, 1, 256:512],
                        tri[:, 0:256], op=MUL)
            return unit

        def att_av(gi, h, p):
            """attn-out accumulation for the pair: two DoubleRow matmuls with
            v-residual stationary planes and stride-0 u moving planes."""
            npairs = 2 * (gi + 1)

            def unit():
                if p == 0:
                    avtiles[gi, h] = ps_av.tile([DH + 1, TS], F32, tag="av",
                                                name=f"av_{gi}_{h}")
                av = avtiles[gi, h]
                u = utiles[gi, h, p]
                if p == npairs - 1:
                    utiles.pop((gi, h, p))
                u3 = u[:].rearrange("q (j i) -> q j i", j=2)
                for jj in range(2):
                    jt = 2 * p + jj
                    d = jt * 128 - gi * TS
                    c0 = max(d, 0)
                    jts, jb = jt // 4, jt % 4
                    vst = _plane(Vt[jts], jb * VB + h * (DH + 1),
                                 stride=VS, m=DH + 1)
                    um = u3[:, jj, c0:TS].unsqueeze(1).broadcast_to(
                        (128, 2, TS - c0))
                    nc.tensor.matmul(av[:, c0:TS], vst, um,
                                     start=(jt == 0), stop=(jt == 2 * npairs - 1),
                                     perf_mode=DR)
            return unit

        def att_norm(gi, h):
            """softmax normalization: 1/denominator broadcast-multiplied into
            aT, straight from the av psum tile."""
            hp, p0 = h // 2, 64 * (h % 2)

            def unit():
                av = avtiles.pop((gi, h))
                r = rows.tile([1, TS], F32, tag="r", name=f"r_{gi}_{h}")
                nc.vector.reciprocal(r[:], av[DH:DH + 1, :])
                rbc = rbcp.tile([DH, TS], F32, tag="rbc", name=f"rbc_{gi}_{h}")
                nc.gpsimd.partition_broadcast(rbc[:], r[:])
                nc.vector.tensor_tensor(
                    aT[hp, gi][p0:p0 + 64, :], av[0:DH, :], rbc[:], op=MUL)
            return unit

        def proj_unit(tt, tail=False, act_copy=False):
            gi = tt // 4

            def unit():
                o = outp.tile([128, 2 * TS], F16, tag="o", name=f"o_{tt}")
                if tail:
                    # attention psum pools are free by now; one 2-bank tile
                    # holds both nt accumulation groups -> single merged copy
                    ps2 = ps_s.tile([128, 2 * TS], F32, tag="ss",
                                    name=f"pj_{tt}")
                    for nt in range(2):
                        for c in range(2):
                            nc.tensor.matmul(
                                ps2[:, nt * TS:(nt + 1) * TS],
                                aT[c, gi][:, (tt % 4) * 128:(tt % 4 + 1) * 128],
                                wp[:, c * D + nt * TS:c * D + (nt + 1) * TS],
                                start=(c == 0), stop=(c == 1))
                    if act_copy:
                        nc.scalar.copy(o[:], ps2[:])
                    else:
                        nc.vector.tensor_copy(o[:], ps2[:])
                else:
                    for nt in range(2):
                        ps = ps_mm.tile([128, TS], F32, tag="mm",
                                        name=f"pj_{tt}_{nt}")
                        for c in range(2):
                            nc.tensor.matmul(
                                ps[:],
                                aT[c, gi][:, (tt % 4) * 128:(tt % 4 + 1) * 128],
                                wp[:, c * D + nt * TS:c * D + (nt + 1) * TS],
                                start=(c == 0), stop=(c == 1))
                        nc.vector.tensor_copy(o[:, nt * TS:(nt + 1) * TS],
                                              ps[:])
                nc.sync.dma_start(out=out_dram[tt], in_=o[:])
            return unit

        def phase_a_units(ts):
            us = []
            if ts > 0:
                us.append(load_xr(ts))
            # all q/k groups first: they gate the next slice's attention
            for ct in (0, 2, 1, 3):
                us.append(qk_group(ts, ct))
            for sub in range(4):
                us.append(v_group(ts, sub))
            return us

        def boost(u):
            def unit():
                with tc.high_priority():
                    u()
            return unit

        def attention_units(gi, nboost=0):
            """Flat pipeline over (h, pair): av trails its exp by `depth`
            slots so the ACT stream never drains. The first `nboost` sc/exp
            units are emitted at top priority so they preempt leftover
            phase-A filler the moment their inputs are ready."""
            npairs = 2 * (gi + 1)
            seq = [(h, p) for h in range(HG) for p in range(npairs)]
            depth = min(3, npairs)
            us = []
            for idx, (h, p) in enumerate(seq):
                sc_u, ex_u = att_sc(gi, h, p), att_exp(gi, h, p)
                if idx < nboost:
                    sc_u, ex_u = boost(sc_u), boost(ex_u)
                us.append(sc_u)
                us.append(ex_u)
                if idx >= depth:
                    ph, pp = seq[idx - depth]
                    us.append(att_av(gi, ph, pp))
                    if pp == npairs - 1:
                        us.append(att_norm(gi, ph))
            for idx in range(len(seq) - depth, len(seq)):
                ph, pp = seq[idx]
                us.append(att_av(gi, ph, pp))
                if pp == npairs - 1:
                    us.append(att_norm(gi, ph))
            return us

        # slice 0: only the ct 0/2 projections must precede attention(0,h0).
        # The rest of phase A is injected into attention(0) at positions that
        # keep every producer emitted before its consumer (tile dependency
        # tracking is emission-order based): v_group(sub) before the first
        # att_av reading that j-block.
        qk_group(0, 0)()
        qk_group(0, 2)()

        def attention0_units():
            a = attention_units(0)
            # flat layout: [sc00,e00, sc01,e01, sc10,e10, av00, sc11,e11,
            #               av01, norm0, ...]
            inject = [(2, qk_group(0, 1)), (4, qk_group(0, 3)),
                      (6, v_group(0, 0)), (6, v_group(0, 1)),
                      (9, v_group(0, 2)), (9, v_group(0, 3))]
            for pos, u in reversed(inject):
                a.insert(pos, u)
            return a

        def place(prim, placed):
            """Insert (fraction, unit) fillers at given fractions of prim."""
            out = list(prim)
            n = len(prim)
            for frac, u in sorted(placed, key=lambda t: -t[0]):
                out.insert(min(int(frac * n), len(out)), u)
            return out

        for gi in range(NTS):
            if gi < NTS - 1:
                ts = gi + 1
                # the next slice's q/k (head-pair 0) gates the exp stream at
                # the gi -> gi+1 boundary: load + project + cast them asap.
                placed = [(0.02, boost(load_xr(ts))),
                          (0.04, boost(qk_group(ts, 0))),
                          (0.07, boost(qk_group(ts, 2))),
                          (0.30, qk_group(ts, 1)),
                          (0.40, qk_group(ts, 3)),
                          (0.50, v_group(ts, 0)), (0.58, v_group(ts, 1)),
                          (0.66, v_group(ts, 2)), (0.74, v_group(ts, 3))]
            else:
                placed = [(0.04 + 0.072 * i, proj_unit(tt))
                          for i, tt in enumerate(range(0, 12))]
            prim = attention0_units() if gi == 0 else attention_units(gi, nboost=2)
            for u in place(prim, placed):
                u()
        for tt in range(12, 16):
            proj_unit(tt, tail=True, act_copy=(tt % 2 == 1))()

    nc.finalize()
    return nc


_NC = None


def _get_nc():
    global _NC
    if _NC is None:
        _NC = build()
    return _NC


def _make_in_maps(x, W_attn, b_attn, W_proj):
    jj = np.arange(128, dtype=np.int64)[:, None]
    ii = np.arange(128, dtype=np.int64)[None, :]
    tri = (jj <= ii).astype(np.float32)
    mask = np.ascontiguousarray(
        np.concatenate([np.zeros((128, 128), np.float32), tri], axis=1))

    shards = []
    for g in range(4):
        q_cols = W_attn[:, g * CV:(g + 1) * CV]
        k_cols = W_attn[:, D + g * CV:D + (g + 1) * CV]
        wqk = np.concatenate([q_cols, k_cols], axis=1)          # [D, 512]
        w1 = (wqk * SW).astype(NP8)
        w2 = (wqk * SW - np.asarray(w1, np.float32)).astype(NP8)
        # [ctpair, 128, (k, resid, ctl, c)]: ct groups {0,2} and {1,3}
        wr = np.stack([w1.reshape(KT, 128, 4, 128),
                       w2.reshape(KT, 128, 4, 128)], axis=1)    # [KT,2r,128,4ct,128]
        wqk8 = np.ascontiguousarray(np.stack(
            [wr[:, :, :, (0, 2), :], wr[:, :, :, (1, 3), :]], axis=0
        ).transpose(0, 3, 1, 2, 4, 5).reshape(2, 128, KT * 4 * 128))
        wvg = np.ascontiguousarray(
            W_attn[:, 2 * D + g * CV:2 * D + (g + 1) * CV].reshape(
                KT, 128, CV).transpose(1, 0, 2).reshape(128, KT * CV)
        ).astype(NPBF)
        wpg = np.ascontiguousarray(
            W_proj[g * CV:(g + 1) * CV, :].reshape(2, 128, D).transpose(
                1, 0, 2).reshape(128, 2 * D))
        bq = b_attn[g * CV:(g + 1) * CV]
        bk = b_attn[D + g * CV:D + (g + 1) * CV]
        bqk = np.ascontiguousarray(
            np.concatenate([bq, bk]).reshape(4, 128).T).astype(np.float32)
        shards.append((wqk8, wvg, wpg, bqk))

    in_maps = []
    for b in range(B):
        xt = x[b].T.reshape(KT, 128, NTS, TS)                   # [k, p, ts, t]
        xT = np.ascontiguousarray(
            xt.transpose(2, 1, 0, 3).reshape(NTS, 128, KT * TS))
        xT8 = xT.astype(NP8)
        xTb = xT.astype(NPBF)
        for g in range(4):
            wqk8, wvg, wpg, bqk = shards[g]
            in_maps.append({
                "xT": xTb, "xT8": xT8, "wqk": wqk8, "wv": wvg, "wp": wpg,
                "bqk": bqk, "mask": mask,
            })
    return in_maps


def run(inputs, trace=False):
    x = np.asarray(inputs["x"], dtype=np.float32)
    W_attn = np.asarray(inputs["W_attn"], dtype=np.float32)
    b_attn = np.asarray(inputs["b_attn"], dtype=np.float32)
    W_proj = np.asarray(inputs["W_proj"], dtype=np.float32)
    b_proj = np.asarray(inputs["b_proj"], dtype=np.float32)

    nc = _get_nc()
    in_maps = _make_in_maps(x, W_attn, b_attn, W_proj)
    res = run_bass_kernel_spmd(nc, in_maps, list(range(8)), trace=trace)

    out = np.zeros((B, T, D), dtype=np.float32)
    for b in range(B):
        for g in range(4):
            out[b] += res.results[b * 4 + g]["out"].reshape(T, D).astype(np.float32)
    # v-bias contributes a constant shift through the value path; b_proj too.
    const = b_attn[2 * D:3 * D] @ W_proj + b_proj
    out += const[None, None, :].astype(np.float32)
    return out, res


def kernel(**inputs):
    out, _ = run(inputs, trace=False)
    return out
